# revision 4
# baseline (speedup 1.0000x reference)
"""Trainium2 Bass kernel: batched channel-attention (Gram-matrix form).

Self-contained: builds the Bass/Tile program, shards the full inputs over
8 NeuronCores (one batch element each), and gathers the full output.

v5 structure (per core, x = one batch element [C, N] fp16):
  The Gram needs X^T subtiles; the tail XT_CNT of them come host-transposed
  (ones-columns baked in), the first PE_SUBS are transposed on the PE from
  native chunks.  Loads INTERLEAVE the two streams [xt, native-chunk, xt,
  ...] so the PE (which grams an xt subtile ~0.17us but needs ~0.29us for a
  transpose+gram subtile) is continuously fed at the DMA arrival rate --
  the Gram finishes ~7us after the last gram byte lands instead of
  serializing a PE-bound transpose phase behind the whole load.
  Native tail chunks (phase-B-only columns) load last, during the algebra
  and phase B; stores overlap them on the shared HBM pipe.

  Algebra: att = W1 G W2^T + rank-1 bias terms (fp32r, centered Gram).
  Softmax folded as A_fin = I + D^{-1} exp(att - max).

  Phase B is pipelined per output half o: softmax(0) -> transpose ->
  y-half-0 matmuls/stores while softmax(1) runs on Vector/Scalar.

DMA discipline: loads on the sync ring in priority order, stores on the
scalar ring.  x/y live in DRAM as [128, 2, N]; I/O fp16.
"""

import bisect
from contextlib import ExitStack

import concourse.bass as bass
import concourse.tile as tile
from concourse import bacc, mybir
from concourse.masks import make_identity

F32 = mybir.dt.float32
F16 = mybir.dt.float16
F32R = mybir.dt.float32r

C = 256
CH = 128  # half of C, = partition count

PE_SUBS = 56            # subtiles transposed on PE (cols 0 .. PE_SUBS*128)
XT_CNT = 128 - PE_SUBS  # host-transposed subtiles (the tail columns)
TRANS_CHUNK = 512       # native chunk size feeding one PE-transpose group run
# xt DMA split: first 7 subtiles, then 13 x 5, interleaved with the chunks
XT_DMA_SPLIT = (7,) + (5,) * 13

# native chunks (cols): 14 x 512 cover the PE-transpose region, the tail
# chunks arrive last (phase B only).
CHUNKS = (512,) * 14 + (3584, 3584, 2048)
N_TRANS_CHUNKS = 14


def build_nc(
    N=16384,
    out_chunks=(2048, 2048, 2048, 2048, 2048, 2048, 2048, 1024, 512, 512),
    cb=4,              # subtiles per batched stash copy
    stash_bufs=3,      # stash tiles of cb subtiles each
    tpsum_bufs=3,      # tp psum tiles (2 banks each)
    attv_bufs=5,
    out_bufs=4,
    warmup=20,
    keepwarm=10,
):
    NSUBS = N // 128
    assert PE_SUBS % cb == 0
    assert sum(CHUNKS) == N
    assert sum(CHUNKS[:N_TRANS_CHUNKS]) == PE_SUBS * 128
    assert sum(XT_DMA_SPLIT) == XT_CNT
    N_ = N
    nc = bacc.Bacc(None, target_bir_lowering=False)

    # x / y as [128, 2, N]: partition p holds channels p and p+128.
    x = nc.dram_tensor("x", [CH, 2, N], F16, kind="ExternalInput")
    # host-transposed tail subtiles, ones-columns pre-baked
    xt = nc.dram_tensor("xt", [CH, XT_CNT, C + 2], F16, kind="ExternalInput")
    wp = nc.dram_tensor("wp", [CH, 4, C], F32R, kind="ExternalInput")
    bp = nc.dram_tensor("bp", [1, 2, C], F32R, kind="ExternalInput")
    y = nc.dram_tensor("y", [CH, 2, N], F16, kind="ExternalOutput")

    starts = []
    pos = 0
    for w in CHUNKS:
        starts.append(pos)
        pos += w

    with tile.TileContext(nc) as tc, ExitStack() as ctx:
        consts = ctx.enter_context(tc.tile_pool(name="consts", bufs=1))
        xfp = ctx.enter_context(tc.tile_pool(name="xf", bufs=1))
        small = ctx.enter_context(tc.tile_pool(name="small", bufs=1))

        ident = consts.tile([128, 128], F16, name="ident", tag="ident")
        make_identity(nc, ident[:])
        ident_f = consts.tile([128, 128], F32, name="ident_f", tag="ident_f")
        make_identity(nc, ident_f[:])
        ident_r = consts.tile([128, 128], F32R, name="ident_r", tag="ident_r")
        nc.vector.tensor_copy(ident_r[:], ident_f[:])
        # per-half identity blocks for the folded softmax: I at columns osl
        identI = [consts.tile([CH, C], F16, name=f"idI{o}", tag=f"idI{o}") for o in range(2)]
        for o in range(2):
            nc.vector.memset(identI[o][:, :], 0.0)
            nc.vector.tensor_copy(identI[o][:, o * CH:(o + 1) * CH], ident[:])

        # --- PE warm-up: dependency-free matmuls un-throttle the HAM clock
        # while the first xt tranche is still in flight ---
        with tc.tile_pool(name="psum_w", bufs=1, space="PSUM") as pw:
            trash = pw.tile([128, 128], F32, name="trash", tag="trash")
            for _ in range(warmup):
                nc.tensor.matmul(trash[:], ident[:], ident[:], start=True, stop=True)

        # --- input DMAs, all on the sync ring, interleaved priority order ---
        xfc = [None] * len(CHUNKS)
        for j in range(len(CHUNKS)):
            xfc[j] = xfp.tile([CH, 2, CHUNKS[j]], F16, name=f"xf{j}", tag=f"xf{j}")
        xt_sb = []
        k0 = 0
        for i, kn in enumerate(XT_DMA_SPLIT):
            t = xfp.tile([CH, kn, C + 2], F16, name=f"xt{k0}", tag=f"xt{k0}")
            nc.sync.dma_start(t[:, :, :], xt[:, k0:k0 + kn, :])
            xt_sb.append((k0, kn, t))
            k0 += kn
            # interleave: one transpose-feeding chunk after each xt tranche
            if i < N_TRANS_CHUNKS:
                sl = slice(starts[i], starts[i] + CHUNKS[i])
                nc.sync.dma_start(xfc[i][:, :, :], x[:, :, sl])
        # weights + biases (needed at algebra time)
        wsb = consts.tile([CH, 4, C], F32R, name="wsb", tag="wsb")
        nc.sync.dma_start(wsb[:, :, :], wp[:, :, :])
        w1_sb = [wsb[:, 2 * h, :] for h in range(2)]
        w2_sb = [wsb[:, 2 * h + 1, :] for h in range(2)]
        bsb = small.tile([1, 2, C], F32R, name="bsb", tag="bsb")
        nc.sync.dma_start(bsb[:, :, :], bp[:, :, :])
        b1_row = bsb[:, 0, :]
        b2_row = bsb[:, 1, :]
        # native tail chunks (phase B only)
        for j in range(N_TRANS_CHUNKS, len(CHUNKS)):
            sl = slice(starts[j], starts[j] + CHUNKS[j])
            nc.sync.dma_start(xfc[j][:, :, :], x[:, :, sl])

        def xf_slice(h, lo, width):
            """AP for X[h-half][:, lo:lo+width]; must lie inside one chunk."""
            j = bisect.bisect_right(starts, lo) - 1
            off = lo - starts[j]
            assert off + width <= CHUNKS[j], (lo, width, j)
            return xfc[j][:, h, off:off + width]

        def xt_slice(k):
            """[128, C+2] AP of host-transposed subtile k (global PE_SUBS+k)."""
            for k0, kn, t in xt_sb:
                if k0 <= k < k0 + kn:
                    return t[:, k - k0, :]
            raise AssertionError(k)

        # N * w1t halves for the fp32r diagonal-centering correction term
        nw1t = consts.tile([CH, 2, C], F32R, name="nw1t", tag="nw1t")
        nc.vector.tensor_scalar(
            nw1t[:, :, :], wsb[:, 0:4:2, :].bitcast(F32), float(N_), None,
            op0=mybir.AluOpType.mult,
        )

        # stash: rotating [128, cb, C+2] tiles; ones-columns written once.
        stash = [
            small.tile([128, cb, C + 2], F16, name=f"xts{b}", tag=f"xts{b}")
            for b in range(stash_bufs)
        ]
        for b in range(stash_bufs):
            nc.vector.memset(stash[b][:, :, C:C + 2], 1.0)

        # ---- Phase A: G = xf xf^T (+ s columns), symmetric ----
        # PE work emitted in arrival order: xt tranche grams interleaved
        # with PE-transpose groups, matching the DMA interleave above.
        g_sb = [small.tile([CH, C + 2], F32R, name=f"gsb{h}", tag=f"gsb{h}") for h in range(2)]
        with tc.tile_pool(name="psum_g", bufs=1, space="PSUM") as pg:
            g0 = pg.tile([CH, C + 2], F32, name="g0", tag="g0")
            g1 = pg.tile([CH, CH + 2], F32, name="g1", tag="g1")

            def gram_xt(k):
                xts = xt_slice(k)
                nc.tensor.matmul(
                    g0[:], xts[:, 0:CH], xts[:, :],
                    start=(k == 0), stop=False,
                )
                nc.tensor.matmul(
                    g1[:], xts[:, CH:C], xts[:, CH:C + 2],
                    start=(k == 0), stop=False,
                )

            with tc.tile_pool(name="psum_t", bufs=tpsum_bufs, space="PSUM") as pt:
                xt_done = 0
                for unit, kn in enumerate(XT_DMA_SPLIT):
                    for k in range(xt_done, xt_done + kn):
                        gram_xt(k)
                    xt_done += kn
                    if unit >= N_TRANS_CHUNKS:
                        continue
                    # one PE-transpose group of cb=4 subtiles (= chunk `unit`)
                    grp = unit
                    tp = pt.tile([128, cb, C], F32, name="tps", tag="tps")
                    for kk in range(cb):
                        ns = grp * cb + kk
                        for h in range(2):
                            nc.tensor.matmul(
                                tp[:, kk, h * CH:(h + 1) * CH],
                                xf_slice(h, ns * 128, 128),
                                ident[:],
                                start=True, stop=True,
                            )
                    st = stash[grp % stash_bufs]
                    if grp % 2 == 1:
                        nc.scalar.copy(st[:, :, 0:C], tp[:, :, :])
                    else:
                        nc.vector.tensor_copy(st[:, :, 0:C], tp[:, :, :])
                    for kk in range(cb):
                        ns = grp * cb + kk
                        last = ns == PE_SUBS - 1
                        nc.tensor.matmul(
                            g0[:], st[:, kk, 0:CH], st[:, kk, :],
                            start=False, stop=last,
                        )
                        nc.tensor.matmul(
                            g1[:], st[:, kk, CH:C], st[:, kk, CH:C + 2],
                            start=False, stop=last,
                        )

            # centering: first the cheap s-column copies (unblock w12s),
            # then the big centered copies.  G' = G - N*I.
            nc.vector.tensor_copy(g_sb[0][:, CH:C + 2], g0[:, CH:C + 2])
            nc.vector.tensor_copy(g_sb[1][:, C:C + 2], g1[:, CH:CH + 2])
            nc.vector.scalar_tensor_tensor(
                g_sb[0][:, 0:CH], ident_f[:], -float(N_), g0[:, 0:CH],
                op0=mybir.AluOpType.mult, op1=mybir.AluOpType.add,
            )
            nc.vector.scalar_tensor_tensor(
                g_sb[1][:, CH:C], ident_f[:], -float(N_), g1[:, 0:CH],
                op0=mybir.AluOpType.mult, op1=mybir.AluOpType.add,
            )

        # G10 = G01^T (Gram symmetry), via regular fp32 matmul vs identity
        with tc.tile_pool(name="psum_gt", bufs=1, space="PSUM") as pgt:
            g10 = pgt.tile([128, 128], F32R, name="g10", tag="g10")
            nc.tensor.transpose(g10[:], g_sb[0][:, CH:C], ident_r[:])
            nc.scalar.copy(g_sb[1][:, 0:CH], g10[:])

        # ---- C x C algebra + softmax ----
        negmax = [small.tile([CH, 1], F32, name=f"nm{o}", tag=f"nm{o}") for o in range(2)]
        rowsum = [small.tile([CH, 1], F32, name=f"rs{o}", tag=f"rs{o}") for o in range(2)]
        rowinv = [small.tile([CH, 1], F32, name=f"ri{o}", tag=f"ri{o}") for o in range(2)]
        exp_sb = [small.tile([CH, C], F16, name=f"exp{o}", tag=f"exp{o}") for o in range(2)]
        fin_sb = [small.tile([CH, C], F16, name=f"fin{o}", tag=f"fin{o}") for o in range(2)]

        with tc.tile_pool(name="psum_alg", bufs=1, space="PSUM") as pa:
            w12s_ps = pa.tile([2, 2 * C], F32, name="w12s", tag="w12s")
            for h in range(2):
                nc.tensor.matmul(
                    w12s_ps[:], g_sb[h][:, C:C + 2], wsb[:, 2 * h:2 * h + 2, :],
                    start=(h == 0), stop=(h == 1),
                )
            w1s_row = small.tile([1, C], F32R, name="w1sr", tag="w1sr")
            w2sn_row = small.tile([1, C], F32R, name="w2snr", tag="w2snr")
            nc.vector.tensor_copy(w1s_row[:], w12s_ps[0:1, 0:C])
            nc.vector.scalar_tensor_tensor(
                w2sn_row[:], b2_row.bitcast(F32), float(N),
                w12s_ps[0:1, C:2 * C],
                op0=mybir.AluOpType.mult, op1=mybir.AluOpType.add,
            )

            u_ps = [pa.tile([CH, C], F32, name=f"u{d}", tag=f"u{d}") for d in range(2)]
            for d in range(2):
                for h in range(2):
                    nc.tensor.matmul(
                        u_ps[d][:],
                        g_sb[h][:, d * CH:(d + 1) * CH],
                        w1_sb[h],
                        start=(h == 0), stop=(h == 1),
                    )
            u_sb = [small.tile([CH, C], F32R, name=f"usb{d}", tag=f"usb{d}") for d in range(2)]
            for d in range(2):
                nc.vector.tensor_copy(u_sb[d][:], u_ps[d][:])

            att_ps = [pa.tile([CH, C], F32, name=f"att{o}", tag=f"att{o}") for o in range(2)]
            for o in range(2):
                osl = slice(o * CH, (o + 1) * CH)
                for h in range(2):
                    nc.tensor.matmul(
                        att_ps[o][:], nw1t[:, h, osl], w2_sb[h],
                        start=(h == 0), stop=False,
                    )
                for d in range(2):
                    nc.tensor.matmul(
                        att_ps[o][:], u_sb[d][:, osl], w2_sb[d],
                        start=False, stop=False,
                    )
                nc.tensor.matmul(
                    att_ps[o][:], w1s_row[:, osl], b2_row,
                    start=False, stop=False,
                )
                nc.tensor.matmul(
                    att_ps[o][:], b1_row[:, osl], w2sn_row[:],
                    start=False, stop=True,
                )

            # PE keep-warm during the softmax chain (reuses the retired
            # w12s_ps bank, WAR-ordered after the two row copies)
            for _ in range(keepwarm):
                nc.tensor.matmul(w12s_ps[:, 0:CH], ident[:, 0:2], ident[:], start=True, stop=True)

            # ---- softmax, folded: A_fin = I + exp(att - max) / rowsum ----
            def softmax(o):
                nc.vector.reduce_max(
                    negmax[o][:], att_ps[o][:], axis=mybir.AxisListType.X,
                    negate=True,
                )
                nc.scalar.activation(
                    exp_sb[o][:], att_ps[o][:],
                    mybir.ActivationFunctionType.Exp,
                    bias=negmax[o][:], scale=1.0,
                    accum_out=rowsum[o][:],
                )
                nc.vector.reciprocal(rowinv[o][:], rowsum[o][:])
                nc.vector.scalar_tensor_tensor(
                    fin_sb[o][:], exp_sb[o][:], rowinv[o][:], identI[o][:],
                    op0=mybir.AluOpType.mult, op1=mybir.AluOpType.add,
                )

            softmax(0)
            softmax(1)

        # ---- Phase B, pipelined per output half o ----
        # attT(o)[d] = fin_sb[o][:, d-half]^T; y(o) = attT(o)^T @ X.
        assert sum(out_chunks) == N
        ostarts = []
        p_ = 0
        for w_ in out_chunks:
            ostarts.append(p_)
            p_ += w_
        attt_sb = [
            small.tile([CH, 2, CH], F16, name=f"att_sb{o}", tag=f"att_sb{o}")
            for o in range(2)
        ]
        evac_idx = 0
        with tc.tile_pool(name="psum_tr", bufs=2, space="PSUM") as ptr, \
             tc.tile_pool(name="psum_b", bufs=attv_bufs, space="PSUM") as pb, \
             tc.tile_pool(name="outp", bufs=out_bufs) as op:
            for o in range(2):
                # transpose fin_sb[o] halves -> attT with d on partitions
                tpo = ptr.tile([CH, 2, CH], F32, name="tpo", tag="tpo")
                for d in range(2):
                    nc.tensor.matmul(
                        tpo[:, d, :],
                        fin_sb[o][:, d * CH:(d + 1) * CH],
                        ident[:],
                        start=True, stop=True,
                    )
                nc.scalar.copy(attt_sb[o][:, :, :], tpo[:, :, :])
                for j, oc in enumerate(out_chunks):
                    ob = op.tile([CH, 2048], F16, name="ob", tag="ob")
                    for a0 in range(0, oc, 512):
                        aw = min(512, oc - a0)
                        av = pb.tile([CH, 512], F32, name="av", tag="av")
                        for d in range(2):
                            nc.tensor.matmul(
                                av[:, 0:aw],
                                attt_sb[o][:, d, :],
                                xf_slice(d, ostarts[j] + a0, aw),
                                start=(d == 0), stop=(d == 1),
                            )
                        if evac_idx % 2 == 1:
                            nc.scalar.copy(ob[:, a0:a0 + aw], av[:, 0:aw])
                        else:
                            nc.vector.tensor_copy(ob[:, a0:a0 + aw], av[:, 0:aw])
                        evac_idx += 1
                    nc.scalar.dma_start(
                        y[:, o, ostarts[j]:ostarts[j] + oc], ob[:, 0:oc]
                    )

    nc.compile()
    return nc


# ---------------------------------------------------------------------------
# Host-side entry point: shard batch over the 8 NeuronCores, run, gather.
# ---------------------------------------------------------------------------

import numpy as np

_NC_CACHE = {}


def _get_nc():
    if "nc" not in _NC_CACHE:
        _NC_CACHE["nc"] = build_nc()
    return _NC_CACHE["nc"]


def make_in_maps(x, w1, b1, w2, b2):
    """Shard + marshal full inputs into per-core input maps (fp16 x)."""
    x = np.asarray(x)
    B, C_, H, W = x.shape
    N = H * W
    xb16 = x.reshape(B, C_, N).astype(np.float16)
    # [B, C, N] -> [B, 128, 2, N]: partition p holds channels p and p+128
    xb = np.ascontiguousarray(xb16.reshape(B, 2, CH, N).transpose(0, 2, 1, 3))
    # host-transposed tail subtiles with pre-baked ones-columns:
    # xt[b, p, k, c] = x[b, c, PE_SUBS*128 + 128k + p]; c in [C, C+2) -> 1
    n0 = PE_SUBS * 128
    tr = xb16[:, :, n0:].reshape(B, C_, XT_CNT, CH).transpose(0, 3, 2, 1)
    xtp = np.ones((B, CH, XT_CNT, C_ + 2), dtype=np.float16)
    xtp[:, :, :, 0:C_] = tr
    xtp = np.ascontiguousarray(xtp)
    w1t = np.asarray(w1, dtype=np.float32).T
    w2t = np.asarray(w2, dtype=np.float32).T
    wp = np.ascontiguousarray(
        np.stack([w1t[0:CH], w2t[0:CH], w1t[CH:C_], w2t[CH:C_]], axis=1)
    )  # [128, 4, C]
    bpk = np.ascontiguousarray(
        np.stack(
            [np.asarray(b1, np.float32), np.asarray(b2, np.float32)], axis=0
        ).reshape(1, 2, C_)
    )
    return [
        {"x": xb[i], "xt": xtp[i], "wp": wp, "bp": bpk}
        for i in range(B)
    ]


def kernel(x, w1, b1, w2, b2):
    """Channel-attention forward for x:(8,256,128,128); returns same shape.

    Data-parallel over the batch: one batch element per NeuronCore.
    """
    from concourse.bass_utils import run_bass_kernel_spmd

    x = np.asarray(x)
    B, C_, H, W = x.shape
    N = H * W
    nc = _get_nc()
    in_maps = make_in_maps(x, w1, b1, w2, b2)
    res = run_bass_kernel_spmd(nc, in_maps, core_ids=list(range(B)))
    out = np.stack(
        [
            res.results[i]["y"].astype(np.float32).transpose(1, 0, 2).reshape(C_, N)
            for i in range(B)
        ],
        axis=0,
    )
    return out.reshape(B, C_, H, W)


# revision 8
# speedup vs baseline: 1.0454x; 1.0454x over previous
"""Trainium2 Bass kernel: batched channel-attention (Gram-matrix form).

Self-contained: builds the Bass/Tile program, shards the full inputs over
8 NeuronCores (one batch element each), and gathers the full output.

v5.1 structure (per core, x = one batch element [C, N] fp16):
  The Gram needs X^T subtiles; the tail XT_CNT of them come host-transposed
  (ones-columns baked in), the first PE_SUBS are transposed on the PE from
  native chunks.  Loads INTERLEAVE the two streams [xt, native-chunk, ...]
  and the Gram is emitted in the same order, finely alternating cheap xt
  subtile grams (~0.17us PE) with expensive transpose+gram groups (~0.3us
  PE each subtile) so the PE never stalls on a stash copy and never runs
  dry of arrived data.  Native tail chunks (phase-B-only columns) load
  last, overlapping the algebra and phase B.

  Algebra: att = W1 G W2^T + rank-1 bias terms (fp32r, centered Gram);
  the nw1t rank-1 matmuls run during the centering-copy gap right after
  the Gram.  Softmax folded as A_fin = I + D^{-1} exp(att - max).

  Phase B is pipelined per output half o: softmax(0) -> transpose ->
  y-half-0 matmuls/stores while softmax(1) runs on Vector/Scalar.
  PSUM evacuation alternates Vector/Scalar; stores trigger from the sync
  queue (idle after the loads) so the Scalar queue stays evac-only.

DMA discipline: loads and stores on the sync ring (stores queue behind
the tail loads, which complete before phase B produces output anyway);
x/y live in DRAM as [128, 2, N]; I/O fp16.
"""

import bisect
from contextlib import ExitStack

import concourse.bass as bass
import concourse.tile as tile
from concourse import bacc, mybir
from concourse.masks import make_identity

F32 = mybir.dt.float32
F16 = mybir.dt.float16
F32R = mybir.dt.float32r

C = 256
CH = 128  # half of C, = partition count

PE_SUBS = 56            # subtiles transposed on PE (cols 0 .. PE_SUBS*128)
XT_CNT = 128 - PE_SUBS  # host-transposed subtiles (the tail columns)
# 7 interleave units: each = one xt DMA + one 1024-col native chunk
XT_DMA_SPLIT = (12,) + (10,) * 6

# native chunks (cols): 7 x 1024 cover the PE-transpose region, the tail
# chunks arrive last (phase B only).
CHUNKS = (1024,) * 7 + (3584, 3584, 2048)
N_TRANS_CHUNKS = 7


def build_nc(
    N=16384,
    out_chunks=(2048, 2048, 2048, 2048, 2048, 2048, 2048, 1024, 512, 512),
    cb=4,              # subtiles per batched stash copy
    stash_bufs=3,      # stash tiles of cb subtiles each
    tpsum_bufs=2,      # tp psum tiles (2 banks each)
    attv_bufs=6,
    out_bufs=4,
    warmup=20,
    keepwarm=12,
):
    NSUBS = N // 128
    assert PE_SUBS % cb == 0
    assert sum(CHUNKS) == N
    assert sum(CHUNKS[:N_TRANS_CHUNKS]) == PE_SUBS * 128
    assert sum(XT_DMA_SPLIT) == XT_CNT
    N_ = N
    nc = bacc.Bacc(None, target_bir_lowering=False)

    # x / y as [128, 2, N]: partition p holds channels p and p+128.
    x = nc.dram_tensor("x", [CH, 2, N], F16, kind="ExternalInput")
    # host-transposed tail subtiles, ones-columns pre-baked
    xt = nc.dram_tensor("xt", [CH, XT_CNT, C + 2], F16, kind="ExternalInput")
    wp = nc.dram_tensor("wp", [CH, 4, C], F32R, kind="ExternalInput")
    bp = nc.dram_tensor("bp", [1, 2, C], F32R, kind="ExternalInput")
    y = nc.dram_tensor("y", [CH, 2, N], F16, kind="ExternalOutput")

    starts = []
    pos = 0
    for w in CHUNKS:
        starts.append(pos)
        pos += w

    with tile.TileContext(nc) as tc, ExitStack() as ctx:
        consts = ctx.enter_context(tc.tile_pool(name="consts", bufs=1))
        xfp = ctx.enter_context(tc.tile_pool(name="xf", bufs=1))
        small = ctx.enter_context(tc.tile_pool(name="small", bufs=1))

        ident = consts.tile([128, 128], F16, name="ident", tag="ident")
        make_identity(nc, ident[:])
        ident_f = consts.tile([128, 128], F32, name="ident_f", tag="ident_f")
        make_identity(nc, ident_f[:])
        ident_r = consts.tile([128, 128], F32R, name="ident_r", tag="ident_r")
        nc.vector.tensor_copy(ident_r[:], ident_f[:])
        # per-half identity blocks for the folded softmax: I at columns osl
        identI = [consts.tile([CH, C], F16, name=f"idI{o}", tag=f"idI{o}") for o in range(2)]
        for o in range(2):
            nc.vector.memset(identI[o][:, :], 0.0)
            nc.vector.tensor_copy(identI[o][:, o * CH:(o + 1) * CH], ident[:])

        # --- PE warm-up: dependency-free matmuls un-throttle the HAM clock
        # while the first xt tranche is still in flight ---
        with tc.tile_pool(name="psum_w", bufs=1, space="PSUM") as pw:
            trash = pw.tile([128, 128], F32, name="trash", tag="trash")
            for _ in range(warmup):
                nc.tensor.matmul(trash[:], ident[:], ident[:], start=True, stop=True)

        # --- input DMAs, all on the sync ring, interleaved priority order ---
        xfc = [None] * len(CHUNKS)
        for j in range(len(CHUNKS)):
            xfc[j] = xfp.tile([CH, 2, CHUNKS[j]], F16, name=f"xf{j}", tag=f"xf{j}")
        xt_sb = []
        k0 = 0
        for i, kn in enumerate(XT_DMA_SPLIT):
            t = xfp.tile([CH, kn, C + 2], F16, name=f"xt{k0}", tag=f"xt{k0}")
            nc.sync.dma_start(t[:, :, :], xt[:, k0:k0 + kn, :])
            xt_sb.append((k0, kn, t))
            k0 += kn
            # interleave: one transpose-feeding chunk after each xt tranche
            sl = slice(starts[i], starts[i] + CHUNKS[i])
            nc.sync.dma_start(xfc[i][:, :, :], x[:, :, sl])
        # weights + biases (needed at algebra time)
        wsb = consts.tile([CH, 4, C], F32R, name="wsb", tag="wsb")
        nc.sync.dma_start(wsb[:, :, :], wp[:, :, :])
        w1_sb = [wsb[:, 2 * h, :] for h in range(2)]
        w2_sb = [wsb[:, 2 * h + 1, :] for h in range(2)]
        bsb = small.tile([1, 2, C], F32R, name="bsb", tag="bsb")
        nc.sync.dma_start(bsb[:, :, :], bp[:, :, :])
        b1_row = bsb[:, 0, :]
        b2_row = bsb[:, 1, :]
        # native tail chunks (phase B only)
        for j in range(N_TRANS_CHUNKS, len(CHUNKS)):
            sl = slice(starts[j], starts[j] + CHUNKS[j])
            nc.sync.dma_start(xfc[j][:, :, :], x[:, :, sl])

        def xf_slice(h, lo, width):
            """AP for X[h-half][:, lo:lo+width]; must lie inside one chunk."""
            j = bisect.bisect_right(starts, lo) - 1
            off = lo - starts[j]
            assert off + width <= CHUNKS[j], (lo, width, j)
            return xfc[j][:, h, off:off + width]

        def xt_slice(k):
            """[128, C+2] AP of host-transposed subtile k (global PE_SUBS+k)."""
            for k0, kn, t in xt_sb:
                if k0 <= k < k0 + kn:
                    return t[:, k - k0, :]
            raise AssertionError(k)

        # N * w1t halves for the fp32r diagonal-centering correction term
        nw1t = consts.tile([CH, 2, C], F32R, name="nw1t", tag="nw1t")
        nc.vector.tensor_scalar(
            nw1t[:, :, :], wsb[:, 0:4:2, :].bitcast(F32), float(N_), None,
            op0=mybir.AluOpType.mult,
        )

        # stash: rotating [128, cb, C+2] tiles; ones-columns written once.
        stash = [
            small.tile([128, cb, C + 2], F16, name=f"xts{b}", tag=f"xts{b}")
            for b in range(stash_bufs)
        ]
        for b in range(stash_bufs):
            nc.vector.memset(stash[b][:, :, C:C + 2], 1.0)

        # ---- Phase A: G = xf xf^T (+ s columns), symmetric ----
        # Manual PSUM pool lifetimes (pools grab their full complement at
        # the alloc boundary): pg(2)+pt(6) = 8 banks during the Gram; pt
        # releases before pa(5) allocates; pg releases before the algebra.
        g_sb = [small.tile([CH, C + 2], F32R, name=f"gsb{h}", tag=f"gsb{h}") for h in range(2)]
        if True:
            pa_att = tc.alloc_tile_pool(name="psum_att", bufs=1, space="PSUM")
            pg = tc.alloc_tile_pool(name="psum_g", bufs=1, space="PSUM")
            g0 = pg.tile([CH, C + 2], F32, name="g0", tag="g0")
            g1 = pg.tile([CH, CH + 2], F32, name="g1", tag="g1")

            def gram_xt(k):
                xts = xt_slice(k)
                nc.tensor.matmul(
                    g0[:], xts[:, 0:CH], xts[:, :],
                    start=(k == 0), stop=False,
                )
                nc.tensor.matmul(
                    g1[:], xts[:, CH:C], xts[:, CH:C + 2],
                    start=(k == 0), stop=False,
                )

            pt = tc.alloc_tile_pool(name="psum_t", bufs=tpsum_bufs, space="PSUM")

            def trans_grp(grp):
                tp = pt.tile([128, cb, C], F32, name="tps", tag="tps")
                for kk in range(cb):
                    ns = grp * cb + kk
                    for h in range(2):
                        nc.tensor.matmul(
                            tp[:, kk, h * CH:(h + 1) * CH],
                            xf_slice(h, ns * 128, 128),
                            ident[:],
                            start=True, stop=True,
                        )
                st = stash[grp % stash_bufs]
                if grp % 2 == 1:
                    nc.scalar.copy(st[:, :, 0:C], tp[:, :, :])
                else:
                    nc.vector.tensor_copy(st[:, :, 0:C], tp[:, :, :])
                for kk in range(cb):
                    ns = grp * cb + kk
                    last = ns == PE_SUBS - 1
                    nc.tensor.matmul(
                        g0[:], st[:, kk, 0:CH], st[:, kk, :],
                        start=False, stop=last,
                    )
                    nc.tensor.matmul(
                        g1[:], st[:, kk, CH:C], st[:, kk, CH:C + 2],
                        start=False, stop=last,
                    )

            # fine interleave: [xt half, grp, xt half, grp] per unit
            xt_done = 0
            for unit, kn in enumerate(XT_DMA_SPLIT):
                ka = kn // 2
                for k in range(xt_done, xt_done + ka):
                    gram_xt(k)
                trans_grp(2 * unit)
                for k in range(xt_done + ka, xt_done + kn):
                    gram_xt(k)
                trans_grp(2 * unit + 1)
                xt_done += kn

            pt.release()
            # att banks live in pa_att (below pg on the pool stack);
            # rank-1 terms fill the PE gap while Vector runs the
            # centering copies.
            att_ps = [pa_att.tile([CH, C], F32, name=f"att{o}", tag=f"att{o}") for o in range(2)]
            for o in range(2):
                osl = slice(o * CH, (o + 1) * CH)
                for h in range(2):
                    nc.tensor.matmul(
                        att_ps[o][:], nw1t[:, h, osl], w2_sb[h],
                        start=(h == 0), stop=False,
                    )

            # centering: cheap s-column copies first (unblock w12s/G10),
            # then the centered copies.  G' = G - N*I.
            nc.vector.tensor_copy(g_sb[0][:, CH:C + 2], g0[:, CH:C + 2])
            nc.vector.tensor_copy(g_sb[1][:, C:C + 2], g1[:, CH:CH + 2])
            nc.vector.scalar_tensor_tensor(
                g_sb[0][:, 0:CH], ident_f[:], -float(N_), g0[:, 0:CH],
                op0=mybir.AluOpType.mult, op1=mybir.AluOpType.add,
            )
            nc.vector.scalar_tensor_tensor(
                g_sb[1][:, CH:C], ident_f[:], -float(N_), g1[:, 0:CH],
                op0=mybir.AluOpType.mult, op1=mybir.AluOpType.add,
            )
            pg.release()
            pa = tc.alloc_tile_pool(name="psum_alg", bufs=1, space="PSUM")

            # ---- C x C algebra (pg released; pa holds w12s/u) ----
            w12s_ps = pa.tile([2, 2 * C], F32, name="w12s", tag="w12s")
            for h in range(2):
                nc.tensor.matmul(
                    w12s_ps[:], g_sb[h][:, C:C + 2], wsb[:, 2 * h:2 * h + 2, :],
                    start=(h == 0), stop=(h == 1),
                )
            with tc.tile_pool(name="psum_gt", bufs=1, space="PSUM") as pgt:
                g10 = pgt.tile([128, 128], F32R, name="g10", tag="g10")
                nc.tensor.transpose(g10[:], g_sb[0][:, CH:C], ident_r[:])
                nc.scalar.copy(g_sb[1][:, 0:CH], g10[:])

            w1s_row = small.tile([1, C], F32R, name="w1sr", tag="w1sr")
            w2sn_row = small.tile([1, C], F32R, name="w2snr", tag="w2snr")
            nc.vector.tensor_copy(w1s_row[:], w12s_ps[0:1, 0:C])
            nc.vector.scalar_tensor_tensor(
                w2sn_row[:], b2_row.bitcast(F32), float(N),
                w12s_ps[0:1, C:2 * C],
                op0=mybir.AluOpType.mult, op1=mybir.AluOpType.add,
            )

            u_ps = [pa.tile([CH, C], F32, name=f"u{d}", tag=f"u{d}") for d in range(2)]
            for d in range(2):
                for h in range(2):
                    nc.tensor.matmul(
                        u_ps[d][:],
                        g_sb[h][:, d * CH:(d + 1) * CH],
                        w1_sb[h],
                        start=(h == 0), stop=(h == 1),
                    )
            u_sb = [small.tile([CH, C], F32R, name=f"usb{d}", tag=f"usb{d}") for d in range(2)]
            for d in range(2):
                nc.vector.tensor_copy(u_sb[d][:], u_ps[d][:])

            # att tails (nw1t terms already accumulated)
            for o in range(2):
                osl = slice(o * CH, (o + 1) * CH)
                for d in range(2):
                    nc.tensor.matmul(
                        att_ps[o][:], u_sb[d][:, osl], w2_sb[d],
                        start=False, stop=False,
                    )
                nc.tensor.matmul(
                    att_ps[o][:], w1s_row[:, osl], b2_row,
                    start=False, stop=False,
                )
                nc.tensor.matmul(
                    att_ps[o][:], b1_row[:, osl], w2sn_row[:],
                    start=False, stop=True,
                )

            # PE keep-warm while softmax(0) runs (reuses the retired w12s
            # bank, WAR-ordered after the two row copies)
            for _ in range(keepwarm):
                nc.tensor.matmul(w12s_ps[:, 0:CH], ident[:, 0:2], ident[:], start=True, stop=True)

            # ---- softmax, folded: A_fin = I + exp(att - max) / rowsum ----
            negmax = [small.tile([CH, 1], F32, name=f"nm{o}", tag=f"nm{o}") for o in range(2)]
            rowsum = [small.tile([CH, 1], F32, name=f"rs{o}", tag=f"rs{o}") for o in range(2)]
            rowinv = [small.tile([CH, 1], F32, name=f"ri{o}", tag=f"ri{o}") for o in range(2)]
            exp_sb = [small.tile([CH, C], F16, name=f"exp{o}", tag=f"exp{o}") for o in range(2)]
            fin_sb = [small.tile([CH, C], F16, name=f"fin{o}", tag=f"fin{o}") for o in range(2)]

            def softmax(o):
                nc.vector.reduce_max(
                    negmax[o][:], att_ps[o][:], axis=mybir.AxisListType.X,
                    negate=True,
                )
                nc.scalar.activation(
                    exp_sb[o][:], att_ps[o][:],
                    mybir.ActivationFunctionType.Exp,
                    bias=negmax[o][:], scale=1.0,
                    accum_out=rowsum[o][:],
                )
                nc.vector.reciprocal(rowinv[o][:], rowsum[o][:])
                nc.vector.scalar_tensor_tensor(
                    fin_sb[o][:], exp_sb[o][:], rowinv[o][:], identI[o][:],
                    op0=mybir.AluOpType.mult, op1=mybir.AluOpType.add,
                )

            softmax(0)
            # a second keep-warm batch covers the softmax(0) latency chain
            for _ in range(keepwarm):
                nc.tensor.matmul(w12s_ps[:, 0:CH], ident[:, 0:2], ident[:], start=True, stop=True)
            softmax(1)
            pa.release()
            pa_att.release()

        # ---- Phase B, pipelined per output half o ----
        # attT(o)[d] = fin_sb[o][:, d-half]^T; y(o) = attT(o)^T @ X.
        assert sum(out_chunks) == N
        ostarts = []
        p_ = 0
        for w_ in out_chunks:
            ostarts.append(p_)
            p_ += w_
        attt_sb = [
            small.tile([CH, 2, CH], F16, name=f"att_sb{o}", tag=f"att_sb{o}")
            for o in range(2)
        ]
        evac_idx = 0
        with tc.tile_pool(name="psum_tr", bufs=2, space="PSUM") as ptr, \
             tc.tile_pool(name="psum_b", bufs=attv_bufs, space="PSUM") as pb, \
             tc.tile_pool(name="outp", bufs=out_bufs) as op:
            for o in range(2):
                # transpose fin_sb[o] halves -> attT with d on partitions
                tpo = ptr.tile([CH, 2, CH], F32, name="tpo", tag="tpo")
                for d in range(2):
                    nc.tensor.matmul(
                        tpo[:, d, :],
                        fin_sb[o][:, d * CH:(d + 1) * CH],
                        ident[:],
                        start=True, stop=True,
                    )
                nc.scalar.copy(attt_sb[o][:, :, :], tpo[:, :, :])
                for j, oc in enumerate(out_chunks):
                    ob = op.tile([CH, 2048], F16, name="ob", tag="ob")
                    for a0 in range(0, oc, 512):
                        aw = min(512, oc - a0)
                        av = pb.tile([CH, 512], F32, name="av", tag="av")
                        for d in range(2):
                            nc.tensor.matmul(
                                av[:, 0:aw],
                                attt_sb[o][:, d, :],
                                xf_slice(d, ostarts[j] + a0, aw),
                                start=(d == 0), stop=(d == 1),
                            )
                        if evac_idx % 2 == 1:
                            nc.scalar.copy(ob[:, a0:a0 + aw], av[:, 0:aw])
                        else:
                            nc.vector.tensor_copy(ob[:, a0:a0 + aw], av[:, 0:aw])
                        evac_idx += 1
                    nc.sync.dma_start(
                        y[:, o, ostarts[j]:ostarts[j] + oc], ob[:, 0:oc]
                    )

    nc.compile()
    return nc


# ---------------------------------------------------------------------------
# Host-side entry point: shard batch over the 8 NeuronCores, run, gather.
# ---------------------------------------------------------------------------

import numpy as np

_NC_CACHE = {}


def _get_nc():
    if "nc" not in _NC_CACHE:
        _NC_CACHE["nc"] = build_nc()
    return _NC_CACHE["nc"]


def make_in_maps(x, w1, b1, w2, b2):
    """Shard + marshal full inputs into per-core input maps (fp16 x)."""
    x = np.asarray(x)
    B, C_, H, W = x.shape
    N = H * W
    xb16 = x.reshape(B, C_, N).astype(np.float16)
    # [B, C, N] -> [B, 128, 2, N]: partition p holds channels p and p+128
    xb = np.ascontiguousarray(xb16.reshape(B, 2, CH, N).transpose(0, 2, 1, 3))
    # host-transposed tail subtiles with pre-baked ones-columns:
    # xt[b, p, k, c] = x[b, c, PE_SUBS*128 + 128k + p]; c in [C, C+2) -> 1
    n0 = PE_SUBS * 128
    tr = xb16[:, :, n0:].reshape(B, C_, XT_CNT, CH).transpose(0, 3, 2, 1)
    xtp = np.ones((B, CH, XT_CNT, C_ + 2), dtype=np.float16)
    xtp[:, :, :, 0:C_] = tr
    xtp = np.ascontiguousarray(xtp)
    w1t = np.asarray(w1, dtype=np.float32).T
    w2t = np.asarray(w2, dtype=np.float32).T
    wp = np.ascontiguousarray(
        np.stack([w1t[0:CH], w2t[0:CH], w1t[CH:C_], w2t[CH:C_]], axis=1)
    )  # [128, 4, C]
    bpk = np.ascontiguousarray(
        np.stack(
            [np.asarray(b1, np.float32), np.asarray(b2, np.float32)], axis=0
        ).reshape(1, 2, C_)
    )
    return [
        {"x": xb[i], "xt": xtp[i], "wp": wp, "bp": bpk}
        for i in range(B)
    ]


def kernel(x, w1, b1, w2, b2):
    """Channel-attention forward for x:(8,256,128,128); returns same shape.

    Data-parallel over the batch: one batch element per NeuronCore.
    """
    from concourse.bass_utils import run_bass_kernel_spmd

    x = np.asarray(x)
    B, C_, H, W = x.shape
    N = H * W
    nc = _get_nc()
    in_maps = make_in_maps(x, w1, b1, w2, b2)
    res = run_bass_kernel_spmd(nc, in_maps, core_ids=list(range(B)))
    out = np.stack(
        [
            res.results[i]["y"].astype(np.float32).transpose(1, 0, 2).reshape(C_, N)
            for i in range(B)
        ],
        axis=0,
    )
    return out.reshape(B, C_, H, W)


# revision 9
# speedup vs baseline: 1.1394x; 1.0899x over previous
"""Trainium2 Bass kernel: batched channel-attention (Gram-matrix form).

Self-contained: builds the Bass/Tile program, shards the full inputs over
8 NeuronCores (one batch element each), and gathers the full output.

v5.1 structure (per core, x = one batch element [C, N] fp16):
  The Gram needs X^T subtiles; the tail XT_CNT of them come host-transposed
  (ones-columns baked in), the first PE_SUBS are transposed on the PE from
  native chunks.  Loads INTERLEAVE the two streams [xt, native-chunk, ...]
  and the Gram is emitted in the same order, finely alternating cheap xt
  subtile grams (~0.17us PE) with expensive transpose+gram groups (~0.3us
  PE each subtile) so the PE never stalls on a stash copy and never runs
  dry of arrived data.  Native tail chunks (phase-B-only columns) load
  last, overlapping the algebra and phase B.

  Algebra: att = W1 G W2^T + rank-1 bias terms (fp32r, centered Gram);
  the nw1t rank-1 matmuls run during the centering-copy gap right after
  the Gram.  Softmax folded as A_fin = I + D^{-1} exp(att - max).

  Phase B is pipelined per output half o: softmax(0) -> transpose ->
  y-half-0 matmuls/stores while softmax(1) runs on Vector/Scalar.
  PSUM evacuation alternates Vector/Scalar; stores trigger from the sync
  queue (idle after the loads) so the Scalar queue stays evac-only.

DMA discipline: loads and stores on the sync ring (stores queue behind
the tail loads, which complete before phase B produces output anyway);
x/y live in DRAM as [128, 2, N]; I/O fp16.
"""

import bisect
from contextlib import ExitStack

import concourse.bass as bass
import concourse.tile as tile
from concourse import bacc, mybir
from concourse.masks import make_identity

F32 = mybir.dt.float32
F16 = mybir.dt.float16
F32R = mybir.dt.float32r

C = 256
CH = 128  # half of C, = partition count

PE_SUBS = 56            # subtiles transposed on PE (cols 0 .. PE_SUBS*128)
XT_CNT = 128 - PE_SUBS  # host-transposed subtiles (the tail columns)
# 7 interleave units: each = one xt DMA + one 1024-col native chunk
XT_DMA_SPLIT = (12,) + (10,) * 6

# native chunks (cols): 7 x 1024 cover the PE-transpose region, the tail
# chunks arrive last (phase B only).
CHUNKS = (1024,) * 7 + (3584, 3584, 2048)
N_TRANS_CHUNKS = 7


def build_nc(
    N=16384,
    out_chunks=(2048, 2048, 2048, 2048, 2048, 2048, 2048, 1024, 512, 512),
    cb=4,              # subtiles per batched stash copy
    stash_bufs=3,      # stash tiles of cb subtiles each
    tpsum_bufs=2,      # tp psum tiles (2 banks each)
    attv_bufs=6,
    out_bufs=4,
    warmup=20,
    keepwarm=12,
):
    NSUBS = N // 128
    assert PE_SUBS % cb == 0
    assert sum(CHUNKS) == N
    assert sum(CHUNKS[:N_TRANS_CHUNKS]) == PE_SUBS * 128
    assert sum(XT_DMA_SPLIT) == XT_CNT
    N_ = N
    nc = bacc.Bacc(None, target_bir_lowering=False)

    # x / y as [128, 2, N]: partition p holds channels p and p+128.
    x = nc.dram_tensor("x", [CH, 2, N], F16, kind="ExternalInput")
    # host-transposed tail subtiles, ones-columns pre-baked
    xt = nc.dram_tensor("xt", [CH, XT_CNT, C + 2], F16, kind="ExternalInput")
    wp = nc.dram_tensor("wp", [CH, 4, C], F32R, kind="ExternalInput")
    bp = nc.dram_tensor("bp", [1, 2, C], F32R, kind="ExternalInput")
    y = nc.dram_tensor("y", [CH, 2, N], F16, kind="ExternalOutput")

    starts = []
    pos = 0
    for w in CHUNKS:
        starts.append(pos)
        pos += w

    with tile.TileContext(nc) as tc, ExitStack() as ctx:
        consts = ctx.enter_context(tc.tile_pool(name="consts", bufs=1))
        xfp = ctx.enter_context(tc.tile_pool(name="xf", bufs=1))
        small = ctx.enter_context(tc.tile_pool(name="small", bufs=1))

        ident = consts.tile([128, 128], F16, name="ident", tag="ident")
        make_identity(nc, ident[:])
        ident_f = consts.tile([128, 128], F32, name="ident_f", tag="ident_f")
        make_identity(nc, ident_f[:])
        ident_r = consts.tile([128, 128], F32R, name="ident_r", tag="ident_r")
        nc.vector.tensor_copy(ident_r[:], ident_f[:])
        # per-half identity blocks for the folded softmax: I at columns osl
        identI = [consts.tile([CH, C], F16, name=f"idI{o}", tag=f"idI{o}") for o in range(2)]
        for o in range(2):
            nc.vector.memset(identI[o][:, :], 0.0)
            nc.vector.tensor_copy(identI[o][:, o * CH:(o + 1) * CH], ident[:])

        # --- PE warm-up: dependency-free matmuls un-throttle the HAM clock
        # while the first xt tranche is still in flight ---
        with tc.tile_pool(name="psum_w", bufs=1, space="PSUM") as pw:
            trash = pw.tile([128, 128], F32, name="trash", tag="trash")
            for _ in range(warmup):
                nc.tensor.matmul(trash[:], ident[:], ident[:], start=True, stop=True)

        # --- input DMAs, all on the sync ring, interleaved priority order ---
        xfc = [None] * len(CHUNKS)
        for j in range(len(CHUNKS)):
            xfc[j] = xfp.tile([CH, 2, CHUNKS[j]], F16, name=f"xf{j}", tag=f"xf{j}")
        xt_sb = []
        k0 = 0
        for i, kn in enumerate(XT_DMA_SPLIT):
            t = xfp.tile([CH, kn, C + 2], F16, name=f"xt{k0}", tag=f"xt{k0}")
            nc.sync.dma_start(t[:, :, :], xt[:, k0:k0 + kn, :])
            xt_sb.append((k0, kn, t))
            k0 += kn
            # interleave: one transpose-feeding chunk after each xt tranche
            sl = slice(starts[i], starts[i] + CHUNKS[i])
            nc.sync.dma_start(xfc[i][:, :, :], x[:, :, sl])
        # weights + biases (needed at algebra time)
        wsb = consts.tile([CH, 4, C], F32R, name="wsb", tag="wsb")
        nc.sync.dma_start(wsb[:, :, :], wp[:, :, :])
        w1_sb = [wsb[:, 2 * h, :] for h in range(2)]
        w2_sb = [wsb[:, 2 * h + 1, :] for h in range(2)]
        bsb = small.tile([1, 2, C], F32R, name="bsb", tag="bsb")
        nc.sync.dma_start(bsb[:, :, :], bp[:, :, :])
        b1_row = bsb[:, 0, :]
        b2_row = bsb[:, 1, :]
        # native tail chunks (phase B only)
        for j in range(N_TRANS_CHUNKS, len(CHUNKS)):
            sl = slice(starts[j], starts[j] + CHUNKS[j])
            nc.sync.dma_start(xfc[j][:, :, :], x[:, :, sl])

        def xf_slice(h, lo, width):
            """AP for X[h-half][:, lo:lo+width]; must lie inside one chunk."""
            j = bisect.bisect_right(starts, lo) - 1
            off = lo - starts[j]
            assert off + width <= CHUNKS[j], (lo, width, j)
            return xfc[j][:, h, off:off + width]

        def xt_slice(k):
            """[128, C+2] AP of host-transposed subtile k (global PE_SUBS+k)."""
            for k0, kn, t in xt_sb:
                if k0 <= k < k0 + kn:
                    return t[:, k - k0, :]
            raise AssertionError(k)

        # N * w1t halves for the fp32r diagonal-centering correction term
        nw1t = consts.tile([CH, 2, C], F32R, name="nw1t", tag="nw1t")
        nc.vector.tensor_scalar(
            nw1t[:, :, :], wsb[:, 0:4:2, :].bitcast(F32), float(N_), None,
            op0=mybir.AluOpType.mult,
        )

        # stash: rotating [128, cb, C+2] tiles; ones-columns written once.
        stash = [
            small.tile([128, cb, C + 2], F16, name=f"xts{b}", tag=f"xts{b}")
            for b in range(stash_bufs)
        ]
        for b in range(stash_bufs):
            nc.vector.memset(stash[b][:, :, C:C + 2], 1.0)

        # ---- Phase A: G = xf xf^T (+ s columns), symmetric ----
        # Manual PSUM pool lifetimes (pools grab their full complement at
        # the alloc boundary): pg(2)+pt(6) = 8 banks during the Gram; pt
        # releases before pa(5) allocates; pg releases before the algebra.
        g_sb = [small.tile([CH, C + 2], F32R, name=f"gsb{h}", tag=f"gsb{h}") for h in range(2)]
        if True:
            pa_att = tc.alloc_tile_pool(name="psum_att", bufs=1, space="PSUM")
            pg = tc.alloc_tile_pool(name="psum_g", bufs=1, space="PSUM")
            g0 = pg.tile([CH, C + 2], F32, name="g0", tag="g0")
            g1 = pg.tile([CH, CH + 2], F32, name="g1", tag="g1")

            def gram_xt(k):
                xts = xt_slice(k)
                nc.tensor.matmul(
                    g0[:], xts[:, 0:CH], xts[:, :],
                    start=(k == 0), stop=False,
                )
                nc.tensor.matmul(
                    g1[:], xts[:, CH:C], xts[:, CH:C + 2],
                    start=(k == 0), stop=False,
                )

            pt = tc.alloc_tile_pool(name="psum_t", bufs=tpsum_bufs, space="PSUM")

            def trans_only(grp):
                """Transpose cb subtiles into PSUM and kick the stash copy."""
                tp = pt.tile([128, cb, C], F32, name="tps", tag="tps")
                for kk in range(cb):
                    ns = grp * cb + kk
                    for h in range(2):
                        nc.tensor.matmul(
                            tp[:, kk, h * CH:(h + 1) * CH],
                            xf_slice(h, ns * 128, 128),
                            ident[:],
                            start=True, stop=True,
                        )
                st = stash[grp % stash_bufs]
                if grp % 2 == 1:
                    nc.scalar.copy(st[:, :, 0:C], tp[:, :, :])
                else:
                    nc.vector.tensor_copy(st[:, :, 0:C], tp[:, :, :])

            def gram_stash(grp):
                st = stash[grp % stash_bufs]
                for kk in range(cb):
                    ns = grp * cb + kk
                    last = ns == PE_SUBS - 1
                    nc.tensor.matmul(
                        g0[:], st[:, kk, 0:CH], st[:, kk, :],
                        start=False, stop=last,
                    )
                    nc.tensor.matmul(
                        g1[:], st[:, kk, CH:C], st[:, kk, CH:C + 2],
                        start=False, stop=last,
                    )

            # software-pipelined fine interleave: per half-unit emit
            # [xt grams, transposes(g), stash-grams(g-1)] so the PE never
            # head-blocks on a stash copy (it has ~1.3us of other work
            # queued between a group's copy and its stash-grams).
            xt_done = 0
            pending = None
            for unit, kn in enumerate(XT_DMA_SPLIT):
                ka = kn // 2
                for k in range(xt_done, xt_done + ka):
                    gram_xt(k)
                trans_only(2 * unit)
                if pending is not None:
                    gram_stash(pending)
                for k in range(xt_done + ka, xt_done + kn):
                    gram_xt(k)
                trans_only(2 * unit + 1)
                gram_stash(2 * unit)
                pending = 2 * unit + 1
                xt_done += kn
            gram_stash(pending)

            pt.release()
            # att banks live in pa_att (below pg on the pool stack);
            # rank-1 terms fill the PE gap while Vector runs the
            # centering copies.
            att_ps = [pa_att.tile([CH, C], F32, name=f"att{o}", tag=f"att{o}") for o in range(2)]
            for o in range(2):
                osl = slice(o * CH, (o + 1) * CH)
                for h in range(2):
                    nc.tensor.matmul(
                        att_ps[o][:], nw1t[:, h, osl], w2_sb[h],
                        start=(h == 0), stop=False,
                    )

            # centering: cheap s-column copies first (unblock w12s/G10),
            # then the centered copies.  G' = G - N*I.
            nc.vector.tensor_copy(g_sb[0][:, CH:C + 2], g0[:, CH:C + 2])
            nc.vector.tensor_copy(g_sb[1][:, C:C + 2], g1[:, CH:CH + 2])
            nc.vector.scalar_tensor_tensor(
                g_sb[0][:, 0:CH], ident_f[:], -float(N_), g0[:, 0:CH],
                op0=mybir.AluOpType.mult, op1=mybir.AluOpType.add,
            )
            nc.vector.scalar_tensor_tensor(
                g_sb[1][:, CH:C], ident_f[:], -float(N_), g1[:, 0:CH],
                op0=mybir.AluOpType.mult, op1=mybir.AluOpType.add,
            )
            pg.release()
            pa = tc.alloc_tile_pool(name="psum_alg", bufs=1, space="PSUM")

            # ---- C x C algebra (pg released; pa holds w12s/u) ----
            w12s_ps = pa.tile([2, 2 * C], F32, name="w12s", tag="w12s")
            for h in range(2):
                nc.tensor.matmul(
                    w12s_ps[:], g_sb[h][:, C:C + 2], wsb[:, 2 * h:2 * h + 2, :],
                    start=(h == 0), stop=(h == 1),
                )
            with tc.tile_pool(name="psum_gt", bufs=1, space="PSUM") as pgt:
                g10 = pgt.tile([128, 128], F32R, name="g10", tag="g10")
                nc.tensor.transpose(g10[:], g_sb[0][:, CH:C], ident_r[:])
                nc.scalar.copy(g_sb[1][:, 0:CH], g10[:])

            w1s_row = small.tile([1, C], F32R, name="w1sr", tag="w1sr")
            w2sn_row = small.tile([1, C], F32R, name="w2snr", tag="w2snr")
            nc.vector.tensor_copy(w1s_row[:], w12s_ps[0:1, 0:C])
            nc.vector.scalar_tensor_tensor(
                w2sn_row[:], b2_row.bitcast(F32), float(N),
                w12s_ps[0:1, C:2 * C],
                op0=mybir.AluOpType.mult, op1=mybir.AluOpType.add,
            )

            u_ps = [pa.tile([CH, C], F32, name=f"u{d}", tag=f"u{d}") for d in range(2)]
            for d in range(2):
                for h in range(2):
                    nc.tensor.matmul(
                        u_ps[d][:],
                        g_sb[h][:, d * CH:(d + 1) * CH],
                        w1_sb[h],
                        start=(h == 0), stop=(h == 1),
                    )
            u_sb = [small.tile([CH, C], F32R, name=f"usb{d}", tag=f"usb{d}") for d in range(2)]
            for d in range(2):
                nc.vector.tensor_copy(u_sb[d][:], u_ps[d][:])

            # att tails (nw1t terms already accumulated)
            for o in range(2):
                osl = slice(o * CH, (o + 1) * CH)
                for d in range(2):
                    nc.tensor.matmul(
                        att_ps[o][:], u_sb[d][:, osl], w2_sb[d],
                        start=False, stop=False,
                    )
                nc.tensor.matmul(
                    att_ps[o][:], w1s_row[:, osl], b2_row,
                    start=False, stop=False,
                )
                nc.tensor.matmul(
                    att_ps[o][:], b1_row[:, osl], w2sn_row[:],
                    start=False, stop=True,
                )

            # PE keep-warm while softmax(0) runs (reuses the retired w12s
            # bank, WAR-ordered after the two row copies)
            for _ in range(keepwarm):
                nc.tensor.matmul(w12s_ps[:, 0:CH], ident[:, 0:2], ident[:], start=True, stop=True)

            # ---- softmax, folded: A_fin = I + exp(att - max) / rowsum ----
            negmax = [small.tile([CH, 1], F32, name=f"nm{o}", tag=f"nm{o}") for o in range(2)]
            rowsum = [small.tile([CH, 1], F32, name=f"rs{o}", tag=f"rs{o}") for o in range(2)]
            rowinv = [small.tile([CH, 1], F32, name=f"ri{o}", tag=f"ri{o}") for o in range(2)]
            exp_sb = [small.tile([CH, C], F16, name=f"exp{o}", tag=f"exp{o}") for o in range(2)]
            fin_sb = [small.tile([CH, C], F16, name=f"fin{o}", tag=f"fin{o}") for o in range(2)]

            def softmax(o):
                nc.vector.reduce_max(
                    negmax[o][:], att_ps[o][:], axis=mybir.AxisListType.X,
                    negate=True,
                )
                nc.scalar.activation(
                    exp_sb[o][:], att_ps[o][:],
                    mybir.ActivationFunctionType.Exp,
                    bias=negmax[o][:], scale=1.0,
                    accum_out=rowsum[o][:],
                )
                nc.vector.reciprocal(rowinv[o][:], rowsum[o][:])
                nc.vector.scalar_tensor_tensor(
                    fin_sb[o][:], exp_sb[o][:], rowinv[o][:], identI[o][:],
                    op0=mybir.AluOpType.mult, op1=mybir.AluOpType.add,
                )

            softmax(0)
            # a second keep-warm batch covers the softmax(0) latency chain
            for _ in range(keepwarm):
                nc.tensor.matmul(w12s_ps[:, 0:CH], ident[:, 0:2], ident[:], start=True, stop=True)
            softmax(1)
            pa.release()
            pa_att.release()

        # ---- Phase B, pipelined per output half o ----
        # attT(o)[d] = fin_sb[o][:, d-half]^T; y(o) = attT(o)^T @ X.
        assert sum(out_chunks) == N
        ostarts = []
        p_ = 0
        for w_ in out_chunks:
            ostarts.append(p_)
            p_ += w_
        attt_sb = [
            small.tile([CH, 2, CH], F16, name=f"att_sb{o}", tag=f"att_sb{o}")
            for o in range(2)
        ]
        evac_idx = 0
        with tc.tile_pool(name="psum_tr", bufs=2, space="PSUM") as ptr, \
             tc.tile_pool(name="psum_b", bufs=attv_bufs, space="PSUM") as pb, \
             tc.tile_pool(name="outp", bufs=out_bufs) as op:
            for o in range(2):
                # transpose fin_sb[o] halves -> attT with d on partitions
                tpo = ptr.tile([CH, 2, CH], F32, name="tpo", tag="tpo")
                for d in range(2):
                    nc.tensor.matmul(
                        tpo[:, d, :],
                        fin_sb[o][:, d * CH:(d + 1) * CH],
                        ident[:],
                        start=True, stop=True,
                    )
                nc.scalar.copy(attt_sb[o][:, :, :], tpo[:, :, :])
                for j, oc in enumerate(out_chunks):
                    ob = op.tile([CH, 2048], F16, name="ob", tag="ob")
                    for a0 in range(0, oc, 512):
                        aw = min(512, oc - a0)
                        av = pb.tile([CH, 512], F32, name="av", tag="av")
                        for d in range(2):
                            nc.tensor.matmul(
                                av[:, 0:aw],
                                attt_sb[o][:, d, :],
                                xf_slice(d, ostarts[j] + a0, aw),
                                start=(d == 0), stop=(d == 1),
                            )
                        if evac_idx % 2 == 1:
                            nc.scalar.copy(ob[:, a0:a0 + aw], av[:, 0:aw])
                        else:
                            nc.vector.tensor_copy(ob[:, a0:a0 + aw], av[:, 0:aw])
                        evac_idx += 1
                    nc.sync.dma_start(
                        y[:, o, ostarts[j]:ostarts[j] + oc], ob[:, 0:oc]
                    )

    nc.compile()
    return nc


# ---------------------------------------------------------------------------
# Host-side entry point: shard batch over the 8 NeuronCores, run, gather.
# ---------------------------------------------------------------------------

import numpy as np

_NC_CACHE = {}


def _get_nc():
    if "nc" not in _NC_CACHE:
        _NC_CACHE["nc"] = build_nc()
    return _NC_CACHE["nc"]


def make_in_maps(x, w1, b1, w2, b2):
    """Shard + marshal full inputs into per-core input maps (fp16 x)."""
    x = np.asarray(x)
    B, C_, H, W = x.shape
    N = H * W
    xb16 = x.reshape(B, C_, N).astype(np.float16)
    # [B, C, N] -> [B, 128, 2, N]: partition p holds channels p and p+128
    xb = np.ascontiguousarray(xb16.reshape(B, 2, CH, N).transpose(0, 2, 1, 3))
    # host-transposed tail subtiles with pre-baked ones-columns:
    # xt[b, p, k, c] = x[b, c, PE_SUBS*128 + 128k + p]; c in [C, C+2) -> 1
    n0 = PE_SUBS * 128
    tr = xb16[:, :, n0:].reshape(B, C_, XT_CNT, CH).transpose(0, 3, 2, 1)
    xtp = np.ones((B, CH, XT_CNT, C_ + 2), dtype=np.float16)
    xtp[:, :, :, 0:C_] = tr
    xtp = np.ascontiguousarray(xtp)
    w1t = np.asarray(w1, dtype=np.float32).T
    w2t = np.asarray(w2, dtype=np.float32).T
    wp = np.ascontiguousarray(
        np.stack([w1t[0:CH], w2t[0:CH], w1t[CH:C_], w2t[CH:C_]], axis=1)
    )  # [128, 4, C]
    bpk = np.ascontiguousarray(
        np.stack(
            [np.asarray(b1, np.float32), np.asarray(b2, np.float32)], axis=0
        ).reshape(1, 2, C_)
    )
    return [
        {"x": xb[i], "xt": xtp[i], "wp": wp, "bp": bpk}
        for i in range(B)
    ]


def kernel(x, w1, b1, w2, b2):
    """Channel-attention forward for x:(8,256,128,128); returns same shape.

    Data-parallel over the batch: one batch element per NeuronCore.
    """
    from concourse.bass_utils import run_bass_kernel_spmd

    x = np.asarray(x)
    B, C_, H, W = x.shape
    N = H * W
    nc = _get_nc()
    in_maps = make_in_maps(x, w1, b1, w2, b2)
    res = run_bass_kernel_spmd(nc, in_maps, core_ids=list(range(B)))
    out = np.stack(
        [
            res.results[i]["y"].astype(np.float32).transpose(1, 0, 2).reshape(C_, N)
            for i in range(B)
        ],
        axis=0,
    )
    return out.reshape(B, C_, H, W)


# revision 10
# speedup vs baseline: 1.1547x; 1.0135x over previous
"""Trainium2 Bass kernel: batched channel-attention (Gram-matrix form).

Self-contained: builds the Bass/Tile program, shards the full inputs over
8 NeuronCores (one batch element each), and gathers the full output.

v5.1 structure (per core, x = one batch element [C, N] fp16):
  The Gram needs X^T subtiles; the tail XT_CNT of them come host-transposed
  (ones-columns baked in), the first PE_SUBS are transposed on the PE from
  native chunks.  Loads INTERLEAVE the two streams [xt, native-chunk, ...]
  and the Gram is emitted in the same order, finely alternating cheap xt
  subtile grams (~0.17us PE) with expensive transpose+gram groups (~0.3us
  PE each subtile) so the PE never stalls on a stash copy and never runs
  dry of arrived data.  Native tail chunks (phase-B-only columns) load
  last, overlapping the algebra and phase B.

  Algebra: att = W1 G W2^T + rank-1 bias terms (fp32r, centered Gram);
  the nw1t rank-1 matmuls run during the centering-copy gap right after
  the Gram.  Softmax folded as A_fin = I + D^{-1} exp(att - max).

  Phase B is pipelined per output half o: softmax(0) -> transpose ->
  y-half-0 matmuls/stores while softmax(1) runs on Vector/Scalar.
  PSUM evacuation alternates Vector/Scalar; stores trigger from the sync
  queue (idle after the loads) so the Scalar queue stays evac-only.

DMA discipline: loads and stores on the sync ring (stores queue behind
the tail loads, which complete before phase B produces output anyway);
x/y live in DRAM as [128, 2, N]; I/O fp16.
"""

import bisect
from contextlib import ExitStack

import concourse.bass as bass
import concourse.tile as tile
from concourse import bacc, mybir
from concourse.masks import make_identity

F32 = mybir.dt.float32
F16 = mybir.dt.float16
F32R = mybir.dt.float32r

C = 256
CH = 128  # half of C, = partition count

PE_SUBS = 44            # subtiles transposed on PE (cols 0 .. PE_SUBS*128)
XT_CNT = 128 - PE_SUBS  # host-transposed subtiles (the tail columns)
# 11 interleave units: each = one xt DMA + one 512-col native chunk
XT_DMA_SPLIT = (14,) + (7,) * 10

# native chunks (cols): 11 x 512 cover the PE-transpose region, the tail
# chunks arrive last (phase B only).
CHUNKS = (512,) * 11 + (3584, 3584, 3584)
N_TRANS_CHUNKS = 11


def build_nc(
    N=16384,
    out_chunks=(2048, 2048, 2048, 2048, 2048, 2048, 2048, 1024, 512, 512),
    cb=4,              # subtiles per batched stash copy
    stash_bufs=3,      # stash tiles of cb subtiles each
    tpsum_bufs=2,      # tp psum tiles (2 banks each)
    attv_bufs=6,
    out_bufs=4,
    warmup=20,
    keepwarm=12,
):
    NSUBS = N // 128
    assert PE_SUBS % cb == 0
    assert sum(CHUNKS) == N
    assert sum(CHUNKS[:N_TRANS_CHUNKS]) == PE_SUBS * 128
    assert sum(XT_DMA_SPLIT) == XT_CNT
    N_ = N
    nc = bacc.Bacc(None, target_bir_lowering=False)

    # x / y as [128, 2, N]: partition p holds channels p and p+128.
    x = nc.dram_tensor("x", [CH, 2, N], F16, kind="ExternalInput")
    # host-transposed tail subtiles, ones-columns pre-baked
    xt = nc.dram_tensor("xt", [CH, XT_CNT, C + 2], F16, kind="ExternalInput")
    wp = nc.dram_tensor("wp", [CH, 4, C], F32R, kind="ExternalInput")
    bp = nc.dram_tensor("bp", [1, 2, C], F32R, kind="ExternalInput")
    y = nc.dram_tensor("y", [CH, 2, N], F16, kind="ExternalOutput")

    starts = []
    pos = 0
    for w in CHUNKS:
        starts.append(pos)
        pos += w

    with tile.TileContext(nc) as tc, ExitStack() as ctx:
        consts = ctx.enter_context(tc.tile_pool(name="consts", bufs=1))
        xfp = ctx.enter_context(tc.tile_pool(name="xf", bufs=1))
        small = ctx.enter_context(tc.tile_pool(name="small", bufs=1))

        ident = consts.tile([128, 128], F16, name="ident", tag="ident")
        make_identity(nc, ident[:])
        ident_f = consts.tile([128, 128], F32, name="ident_f", tag="ident_f")
        make_identity(nc, ident_f[:])
        ident_r = consts.tile([128, 128], F32R, name="ident_r", tag="ident_r")
        nc.vector.tensor_copy(ident_r[:], ident_f[:])
        # per-half identity blocks for the folded softmax: I at columns osl
        identI = [consts.tile([CH, C], F16, name=f"idI{o}", tag=f"idI{o}") for o in range(2)]
        for o in range(2):
            nc.vector.memset(identI[o][:, :], 0.0)
            nc.vector.tensor_copy(identI[o][:, o * CH:(o + 1) * CH], ident[:])

        # --- PE warm-up: dependency-free matmuls un-throttle the HAM clock
        # while the first xt tranche is still in flight ---
        with tc.tile_pool(name="psum_w", bufs=1, space="PSUM") as pw:
            trash = pw.tile([128, 128], F32, name="trash", tag="trash")
            for _ in range(warmup):
                nc.tensor.matmul(trash[:], ident[:], ident[:], start=True, stop=True)

        # --- input DMAs, all on the sync ring, interleaved priority order ---
        xfc = [None] * len(CHUNKS)
        for j in range(len(CHUNKS)):
            xfc[j] = xfp.tile([CH, 2, CHUNKS[j]], F16, name=f"xf{j}", tag=f"xf{j}")
        xt_sb = []
        k0 = 0
        for i, kn in enumerate(XT_DMA_SPLIT):
            t = xfp.tile([CH, kn, C + 2], F16, name=f"xt{k0}", tag=f"xt{k0}")
            nc.sync.dma_start(t[:, :, :], xt[:, k0:k0 + kn, :])
            xt_sb.append((k0, kn, t))
            k0 += kn
            # interleave: one transpose-feeding chunk after each xt tranche
            sl = slice(starts[i], starts[i] + CHUNKS[i])
            nc.sync.dma_start(xfc[i][:, :, :], x[:, :, sl])
        # weights + biases (needed at algebra time)
        wsb = consts.tile([CH, 4, C], F32R, name="wsb", tag="wsb")
        nc.sync.dma_start(wsb[:, :, :], wp[:, :, :])
        w1_sb = [wsb[:, 2 * h, :] for h in range(2)]
        w2_sb = [wsb[:, 2 * h + 1, :] for h in range(2)]
        bsb = small.tile([1, 2, C], F32R, name="bsb", tag="bsb")
        nc.sync.dma_start(bsb[:, :, :], bp[:, :, :])
        b1_row = bsb[:, 0, :]
        b2_row = bsb[:, 1, :]
        # native tail chunks (phase B only)
        for j in range(N_TRANS_CHUNKS, len(CHUNKS)):
            sl = slice(starts[j], starts[j] + CHUNKS[j])
            nc.sync.dma_start(xfc[j][:, :, :], x[:, :, sl])

        def xf_slice(h, lo, width):
            """AP for X[h-half][:, lo:lo+width]; must lie inside one chunk."""
            j = bisect.bisect_right(starts, lo) - 1
            off = lo - starts[j]
            assert off + width <= CHUNKS[j], (lo, width, j)
            return xfc[j][:, h, off:off + width]

        def xt_slice(k):
            """[128, C+2] AP of host-transposed subtile k (global PE_SUBS+k)."""
            for k0, kn, t in xt_sb:
                if k0 <= k < k0 + kn:
                    return t[:, k - k0, :]
            raise AssertionError(k)

        # N * w1t halves for the fp32r diagonal-centering correction term
        nw1t = consts.tile([CH, 2, C], F32R, name="nw1t", tag="nw1t")
        nc.vector.tensor_scalar(
            nw1t[:, :, :], wsb[:, 0:4:2, :].bitcast(F32), float(N_), None,
            op0=mybir.AluOpType.mult,
        )

        # stash: rotating [128, cb, C+2] tiles; ones-columns written once.
        stash = [
            small.tile([128, cb, C + 2], F16, name=f"xts{b}", tag=f"xts{b}")
            for b in range(stash_bufs)
        ]
        for b in range(stash_bufs):
            nc.vector.memset(stash[b][:, :, C:C + 2], 1.0)

        # ---- Phase A: G = xf xf^T (+ s columns), symmetric ----
        # Manual PSUM pool lifetimes (pools grab their full complement at
        # the alloc boundary): pg(2)+pt(6) = 8 banks during the Gram; pt
        # releases before pa(5) allocates; pg releases before the algebra.
        g_sb = [small.tile([CH, C + 2], F32R, name=f"gsb{h}", tag=f"gsb{h}") for h in range(2)]
        if True:
            pa_att = tc.alloc_tile_pool(name="psum_att", bufs=1, space="PSUM")
            pg = tc.alloc_tile_pool(name="psum_g", bufs=1, space="PSUM")
            g0 = pg.tile([CH, C + 2], F32, name="g0", tag="g0")
            g1 = pg.tile([CH, CH + 2], F32, name="g1", tag="g1")

            def gram_xt(k):
                xts = xt_slice(k)
                nc.tensor.matmul(
                    g0[:], xts[:, 0:CH], xts[:, :],
                    start=(k == 0), stop=False,
                )
                nc.tensor.matmul(
                    g1[:], xts[:, CH:C], xts[:, CH:C + 2],
                    start=(k == 0), stop=False,
                )

            pt = tc.alloc_tile_pool(name="psum_t", bufs=tpsum_bufs, space="PSUM")

            def trans_only(grp):
                """Transpose cb subtiles into PSUM and kick the stash copy."""
                tp = pt.tile([128, cb, C], F32, name="tps", tag="tps")
                for kk in range(cb):
                    ns = grp * cb + kk
                    for h in range(2):
                        nc.tensor.matmul(
                            tp[:, kk, h * CH:(h + 1) * CH],
                            xf_slice(h, ns * 128, 128),
                            ident[:],
                            start=True, stop=True,
                        )
                st = stash[grp % stash_bufs]
                if grp % 2 == 1:
                    nc.scalar.copy(st[:, :, 0:C], tp[:, :, :])
                else:
                    nc.vector.tensor_copy(st[:, :, 0:C], tp[:, :, :])

            def gram_stash(grp):
                st = stash[grp % stash_bufs]
                for kk in range(cb):
                    ns = grp * cb + kk
                    last = ns == PE_SUBS - 1
                    nc.tensor.matmul(
                        g0[:], st[:, kk, 0:CH], st[:, kk, :],
                        start=False, stop=last,
                    )
                    nc.tensor.matmul(
                        g1[:], st[:, kk, CH:C], st[:, kk, CH:C + 2],
                        start=False, stop=last,
                    )

            # software-pipelined fine interleave: per unit emit
            # [xt-half grams, transposes(g), xt-half, stash-grams(g-1)] so
            # the PE never head-blocks on a stash copy (it has ~1.3us of
            # other work queued between a group's copy and its grams).
            xt_done = 0
            pending = None
            for unit, kn in enumerate(XT_DMA_SPLIT):
                ka = kn // 2
                for k in range(xt_done, xt_done + ka):
                    gram_xt(k)
                trans_only(unit)
                for k in range(xt_done + ka, xt_done + kn):
                    gram_xt(k)
                if pending is not None:
                    gram_stash(pending)
                pending = unit
                xt_done += kn
            gram_stash(pending)

            pt.release()
            # att banks live in pa_att (below pg on the pool stack);
            # rank-1 terms fill the PE gap while Vector runs the
            # centering copies.
            att_ps = [pa_att.tile([CH, C], F32, name=f"att{o}", tag=f"att{o}") for o in range(2)]
            for o in range(2):
                osl = slice(o * CH, (o + 1) * CH)
                for h in range(2):
                    nc.tensor.matmul(
                        att_ps[o][:], nw1t[:, h, osl], w2_sb[h],
                        start=(h == 0), stop=False,
                    )

            # centering: cheap s-column copies first (unblock w12s/G10),
            # then the centered copies.  G' = G - N*I.
            nc.vector.tensor_copy(g_sb[0][:, CH:C + 2], g0[:, CH:C + 2])
            nc.vector.tensor_copy(g_sb[1][:, C:C + 2], g1[:, CH:CH + 2])
            nc.vector.scalar_tensor_tensor(
                g_sb[0][:, 0:CH], ident_f[:], -float(N_), g0[:, 0:CH],
                op0=mybir.AluOpType.mult, op1=mybir.AluOpType.add,
            )
            nc.vector.scalar_tensor_tensor(
                g_sb[1][:, CH:C], ident_f[:], -float(N_), g1[:, 0:CH],
                op0=mybir.AluOpType.mult, op1=mybir.AluOpType.add,
            )
            pg.release()
            pa = tc.alloc_tile_pool(name="psum_alg", bufs=1, space="PSUM")

            # ---- C x C algebra (pg released; pa holds w12s/u) ----
            w12s_ps = pa.tile([2, 2 * C], F32, name="w12s", tag="w12s")
            for h in range(2):
                nc.tensor.matmul(
                    w12s_ps[:], g_sb[h][:, C:C + 2], wsb[:, 2 * h:2 * h + 2, :],
                    start=(h == 0), stop=(h == 1),
                )
            with tc.tile_pool(name="psum_gt", bufs=1, space="PSUM") as pgt:
                g10 = pgt.tile([128, 128], F32R, name="g10", tag="g10")
                nc.tensor.transpose(g10[:], g_sb[0][:, CH:C], ident_r[:])
                nc.scalar.copy(g_sb[1][:, 0:CH], g10[:])

            w1s_row = small.tile([1, C], F32R, name="w1sr", tag="w1sr")
            w2sn_row = small.tile([1, C], F32R, name="w2snr", tag="w2snr")
            nc.vector.tensor_copy(w1s_row[:], w12s_ps[0:1, 0:C])
            nc.vector.scalar_tensor_tensor(
                w2sn_row[:], b2_row.bitcast(F32), float(N),
                w12s_ps[0:1, C:2 * C],
                op0=mybir.AluOpType.mult, op1=mybir.AluOpType.add,
            )

            u_ps = [pa.tile([CH, C], F32, name=f"u{d}", tag=f"u{d}") for d in range(2)]
            for d in range(2):
                for h in range(2):
                    nc.tensor.matmul(
                        u_ps[d][:],
                        g_sb[h][:, d * CH:(d + 1) * CH],
                        w1_sb[h],
                        start=(h == 0), stop=(h == 1),
                    )
            u_sb = [small.tile([CH, C], F32R, name=f"usb{d}", tag=f"usb{d}") for d in range(2)]
            for d in range(2):
                nc.vector.tensor_copy(u_sb[d][:], u_ps[d][:])

            # att tails (nw1t terms already accumulated)
            for o in range(2):
                osl = slice(o * CH, (o + 1) * CH)
                for d in range(2):
                    nc.tensor.matmul(
                        att_ps[o][:], u_sb[d][:, osl], w2_sb[d],
                        start=False, stop=False,
                    )
                nc.tensor.matmul(
                    att_ps[o][:], w1s_row[:, osl], b2_row,
                    start=False, stop=False,
                )
                nc.tensor.matmul(
                    att_ps[o][:], b1_row[:, osl], w2sn_row[:],
                    start=False, stop=True,
                )

            # PE keep-warm while softmax(0) runs (reuses the retired w12s
            # bank, WAR-ordered after the two row copies)
            for _ in range(keepwarm):
                nc.tensor.matmul(w12s_ps[:, 0:CH], ident[:, 0:2], ident[:], start=True, stop=True)

            # ---- softmax, folded: A_fin = I + exp(att - max) / rowsum ----
            negmax = [small.tile([CH, 1], F32, name=f"nm{o}", tag=f"nm{o}") for o in range(2)]
            rowsum = [small.tile([CH, 1], F32, name=f"rs{o}", tag=f"rs{o}") for o in range(2)]
            rowinv = [small.tile([CH, 1], F32, name=f"ri{o}", tag=f"ri{o}") for o in range(2)]
            exp_sb = [small.tile([CH, C], F16, name=f"exp{o}", tag=f"exp{o}") for o in range(2)]
            fin_sb = [small.tile([CH, C], F16, name=f"fin{o}", tag=f"fin{o}") for o in range(2)]

            def softmax(o):
                nc.vector.reduce_max(
                    negmax[o][:], att_ps[o][:], axis=mybir.AxisListType.X,
                    negate=True,
                )
                nc.scalar.activation(
                    exp_sb[o][:], att_ps[o][:],
                    mybir.ActivationFunctionType.Exp,
                    bias=negmax[o][:], scale=1.0,
                    accum_out=rowsum[o][:],
                )
                nc.vector.reciprocal(rowinv[o][:], rowsum[o][:])
                nc.vector.scalar_tensor_tensor(
                    fin_sb[o][:], exp_sb[o][:], rowinv[o][:], identI[o][:],
                    op0=mybir.AluOpType.mult, op1=mybir.AluOpType.add,
                )

            softmax(0)
            # a second keep-warm batch covers the softmax(0) latency chain
            for _ in range(keepwarm):
                nc.tensor.matmul(w12s_ps[:, 0:CH], ident[:, 0:2], ident[:], start=True, stop=True)
            softmax(1)
            pa.release()
            pa_att.release()

        # ---- Phase B, pipelined per output half o ----
        # attT(o)[d] = fin_sb[o][:, d-half]^T; y(o) = attT(o)^T @ X.
        assert sum(out_chunks) == N
        ostarts = []
        p_ = 0
        for w_ in out_chunks:
            ostarts.append(p_)
            p_ += w_
        attt_sb = [
            small.tile([CH, 2, CH], F16, name=f"att_sb{o}", tag=f"att_sb{o}")
            for o in range(2)
        ]
        evac_idx = 0
        with tc.tile_pool(name="psum_tr", bufs=2, space="PSUM") as ptr, \
             tc.tile_pool(name="psum_b", bufs=attv_bufs, space="PSUM") as pb, \
             tc.tile_pool(name="outp", bufs=out_bufs) as op:
            # transpose fin_sb halves -> attT with d on partitions (both
            # output halves up front so the o=1 sweep starts seamlessly)
            for o in range(2):
                tpo = ptr.tile([CH, 2, CH], F32, name="tpo", tag="tpo")
                for d in range(2):
                    nc.tensor.matmul(
                        tpo[:, d, :],
                        fin_sb[o][:, d * CH:(d + 1) * CH],
                        ident[:],
                        start=True, stop=True,
                    )
                nc.scalar.copy(attt_sb[o][:, :, :], tpo[:, :, :])
            for o in range(2):
                for j, oc in enumerate(out_chunks):
                    ob = op.tile([CH, 2048], F16, name="ob", tag="ob")
                    for a0 in range(0, oc, 512):
                        aw = min(512, oc - a0)
                        av = pb.tile([CH, 512], F32, name="av", tag="av")
                        for d in range(2):
                            nc.tensor.matmul(
                                av[:, 0:aw],
                                attt_sb[o][:, d, :],
                                xf_slice(d, ostarts[j] + a0, aw),
                                start=(d == 0), stop=(d == 1),
                            )
                        if evac_idx % 2 == 1:
                            nc.scalar.copy(ob[:, a0:a0 + aw], av[:, 0:aw])
                        else:
                            nc.vector.tensor_copy(ob[:, a0:a0 + aw], av[:, 0:aw])
                        evac_idx += 1
                    nc.sync.dma_start(
                        y[:, o, ostarts[j]:ostarts[j] + oc], ob[:, 0:oc]
                    )

    nc.compile()
    return nc


# ---------------------------------------------------------------------------
# Host-side entry point: shard batch over the 8 NeuronCores, run, gather.
# ---------------------------------------------------------------------------

import numpy as np

_NC_CACHE = {}


def _get_nc():
    if "nc" not in _NC_CACHE:
        _NC_CACHE["nc"] = build_nc()
    return _NC_CACHE["nc"]


def make_in_maps(x, w1, b1, w2, b2):
    """Shard + marshal full inputs into per-core input maps (fp16 x)."""
    x = np.asarray(x)
    B, C_, H, W = x.shape
    N = H * W
    xb16 = x.reshape(B, C_, N).astype(np.float16)
    # [B, C, N] -> [B, 128, 2, N]: partition p holds channels p and p+128
    xb = np.ascontiguousarray(xb16.reshape(B, 2, CH, N).transpose(0, 2, 1, 3))
    # host-transposed tail subtiles with pre-baked ones-columns:
    # xt[b, p, k, c] = x[b, c, PE_SUBS*128 + 128k + p]; c in [C, C+2) -> 1
    n0 = PE_SUBS * 128
    tr = xb16[:, :, n0:].reshape(B, C_, XT_CNT, CH).transpose(0, 3, 2, 1)
    xtp = np.ones((B, CH, XT_CNT, C_ + 2), dtype=np.float16)
    xtp[:, :, :, 0:C_] = tr
    xtp = np.ascontiguousarray(xtp)
    w1t = np.asarray(w1, dtype=np.float32).T
    w2t = np.asarray(w2, dtype=np.float32).T
    wp = np.ascontiguousarray(
        np.stack([w1t[0:CH], w2t[0:CH], w1t[CH:C_], w2t[CH:C_]], axis=1)
    )  # [128, 4, C]
    bpk = np.ascontiguousarray(
        np.stack(
            [np.asarray(b1, np.float32), np.asarray(b2, np.float32)], axis=0
        ).reshape(1, 2, C_)
    )
    return [
        {"x": xb[i], "xt": xtp[i], "wp": wp, "bp": bpk}
        for i in range(B)
    ]


def kernel(x, w1, b1, w2, b2):
    """Channel-attention forward for x:(8,256,128,128); returns same shape.

    Data-parallel over the batch: one batch element per NeuronCore.
    """
    from concourse.bass_utils import run_bass_kernel_spmd

    x = np.asarray(x)
    B, C_, H, W = x.shape
    N = H * W
    nc = _get_nc()
    in_maps = make_in_maps(x, w1, b1, w2, b2)
    res = run_bass_kernel_spmd(nc, in_maps, core_ids=list(range(B)))
    out = np.stack(
        [
            res.results[i]["y"].astype(np.float32).transpose(1, 0, 2).reshape(C_, N)
            for i in range(B)
        ],
        axis=0,
    )
    return out.reshape(B, C_, H, W)


# revision 12
# speedup vs baseline: 1.1847x; 1.0260x over previous
"""Trainium2 Bass kernel: batched channel-attention (Gram-matrix form).

Self-contained: builds the Bass/Tile program, shards the full inputs over
8 NeuronCores (one batch element each), and gathers the full output.

v5.1 structure (per core, x = one batch element [C, N] fp16):
  The Gram needs X^T subtiles; the tail XT_CNT of them come host-transposed
  (ones-columns baked in), the first PE_SUBS are transposed on the PE from
  native chunks.  Loads INTERLEAVE the two streams [xt, native-chunk, ...]
  and the Gram is emitted in the same order, finely alternating cheap xt
  subtile grams (~0.17us PE) with expensive transpose+gram groups (~0.3us
  PE each subtile) so the PE never stalls on a stash copy and never runs
  dry of arrived data.  Native tail chunks (phase-B-only columns) load
  last, overlapping the algebra and phase B.

  Algebra: att = W1 G W2^T + rank-1 bias terms (fp32r, centered Gram);
  the nw1t rank-1 matmuls run during the centering-copy gap right after
  the Gram.  Softmax folded as A_fin = I + D^{-1} exp(att - max).

  Phase B is pipelined per output half o: softmax(0) -> transpose ->
  y-half-0 matmuls/stores while softmax(1) runs on Vector/Scalar.
  PSUM evacuation alternates Vector/Scalar; stores trigger from the sync
  queue (idle after the loads) so the Scalar queue stays evac-only.

DMA discipline: loads and stores on the sync ring (stores queue behind
the tail loads, which complete before phase B produces output anyway);
x/y live in DRAM as [128, 2, N]; I/O fp16.
"""

import bisect
from contextlib import ExitStack

import concourse.bass as bass
import concourse.tile as tile
from concourse import bacc, mybir
from concourse.masks import make_identity

F32 = mybir.dt.float32
F16 = mybir.dt.float16
F32R = mybir.dt.float32r

C = 256
CH = 128  # half of C, = partition count

PE_SUBS = 44            # subtiles transposed on PE (cols 0 .. PE_SUBS*128)
XT_CNT = 128 - PE_SUBS  # host-transposed subtiles (the tail columns)
# 11 interleave units: each = one xt DMA + one 512-col native chunk
XT_DMA_SPLIT = (4,) + (8,) * 10

# native chunks (cols): 11 x 512 cover the PE-transpose region, the tail
# chunks arrive last (phase B only).
CHUNKS = (512,) * 11 + (3584, 3584, 3584)
N_TRANS_CHUNKS = 11


def build_nc(
    N=16384,
    out_chunks=(2048, 2048, 2048, 2048, 2048, 2048, 2048, 1024, 512, 512),
    cb=4,              # subtiles per batched stash copy
    stash_bufs=3,      # stash tiles of cb subtiles each
    tpsum_bufs=2,      # tp psum tiles (2 banks each)
    attv_bufs=6,
    out_bufs=6,
    warmup=12,
    keepwarm=12,
):
    NSUBS = N // 128
    assert PE_SUBS % cb == 0
    assert sum(CHUNKS) == N
    assert sum(CHUNKS[:N_TRANS_CHUNKS]) == PE_SUBS * 128
    assert sum(XT_DMA_SPLIT) == XT_CNT
    N_ = N
    nc = bacc.Bacc(None, target_bir_lowering=False)

    # x / y as [128, 2, N]: partition p holds channels p and p+128.
    x = nc.dram_tensor("x", [CH, 2, N], F16, kind="ExternalInput")
    # host-transposed tail subtiles, ones-columns pre-baked
    xt = nc.dram_tensor("xt", [CH, XT_CNT, C + 2], F16, kind="ExternalInput")
    wp = nc.dram_tensor("wp", [CH, 4, C], F32R, kind="ExternalInput")
    bp = nc.dram_tensor("bp", [1, 2, C], F32R, kind="ExternalInput")
    y = nc.dram_tensor("y", [CH, 2, N], F16, kind="ExternalOutput")

    starts = []
    pos = 0
    for w in CHUNKS:
        starts.append(pos)
        pos += w

    with tile.TileContext(nc) as tc, ExitStack() as ctx:
        consts = ctx.enter_context(tc.tile_pool(name="consts", bufs=1))
        xfp = ctx.enter_context(tc.tile_pool(name="xf", bufs=1))
        small = ctx.enter_context(tc.tile_pool(name="small", bufs=1))

        ident = consts.tile([128, 128], F16, name="ident", tag="ident")
        make_identity(nc, ident[:])
        ident_f = consts.tile([128, 128], F32, name="ident_f", tag="ident_f")
        make_identity(nc, ident_f[:])
        ident_r = consts.tile([128, 128], F32R, name="ident_r", tag="ident_r")
        nc.vector.tensor_copy(ident_r[:], ident_f[:])
        # per-half identity blocks for the folded softmax: I at columns osl
        identI = [consts.tile([CH, C], F16, name=f"idI{o}", tag=f"idI{o}") for o in range(2)]
        for o in range(2):
            nc.vector.memset(identI[o][:, :], 0.0)
            nc.vector.tensor_copy(identI[o][:, o * CH:(o + 1) * CH], ident[:])

        # --- PE warm-up: dependency-free matmuls un-throttle the HAM clock
        # while the first xt tranche is still in flight ---
        with tc.tile_pool(name="psum_w", bufs=1, space="PSUM") as pw:
            trash = pw.tile([128, 128], F32, name="trash", tag="trash")
            for _ in range(warmup):
                nc.tensor.matmul(trash[:], ident[:], ident[:], start=True, stop=True)

        # --- input DMAs, all on the sync ring, interleaved priority order ---
        xfc = [None] * len(CHUNKS)
        for j in range(len(CHUNKS)):
            xfc[j] = xfp.tile([CH, 2, CHUNKS[j]], F16, name=f"xf{j}", tag=f"xf{j}")
        xt_sb = []
        k0 = 0
        for i, kn in enumerate(XT_DMA_SPLIT):
            t = xfp.tile([CH, kn, C + 2], F16, name=f"xt{k0}", tag=f"xt{k0}")
            nc.sync.dma_start(t[:, :, :], xt[:, k0:k0 + kn, :])
            xt_sb.append((k0, kn, t))
            k0 += kn
            # interleave: one transpose-feeding chunk after each xt tranche
            sl = slice(starts[i], starts[i] + CHUNKS[i])
            nc.sync.dma_start(xfc[i][:, :, :], x[:, :, sl])
        # weights + biases (needed at algebra time)
        wsb = consts.tile([CH, 4, C], F32R, name="wsb", tag="wsb")
        nc.sync.dma_start(wsb[:, :, :], wp[:, :, :])
        w16 = consts.tile([CH, 4, C], F16, name="w16", tag="w16")
        nc.vector.tensor_copy(w16[:, :, :], wsb[:, :, :].bitcast(F32))
        w1_sb = [w16[:, 2 * h, :] for h in range(2)]
        w2_sb = [w16[:, 2 * h + 1, :] for h in range(2)]
        bsb = small.tile([1, 2, C], F32R, name="bsb", tag="bsb")
        nc.sync.dma_start(bsb[:, :, :], bp[:, :, :])
        b16 = small.tile([1, 2, C], F16, name="b16", tag="b16")
        nc.vector.tensor_copy(b16[:, :, :], bsb[:, :, :].bitcast(F32))
        b1_row = b16[:, 0, :]
        b2_row = b16[:, 1, :]
        # native tail chunks (phase B only)
        for j in range(N_TRANS_CHUNKS, len(CHUNKS)):
            sl = slice(starts[j], starts[j] + CHUNKS[j])
            nc.sync.dma_start(xfc[j][:, :, :], x[:, :, sl])

        def xf_slice(h, lo, width):
            """AP for X[h-half][:, lo:lo+width]; must lie inside one chunk."""
            j = bisect.bisect_right(starts, lo) - 1
            off = lo - starts[j]
            assert off + width <= CHUNKS[j], (lo, width, j)
            return xfc[j][:, h, off:off + width]

        def xt_slice(k):
            """[128, C+2] AP of host-transposed subtile k (global PE_SUBS+k)."""
            for k0, kn, t in xt_sb:
                if k0 <= k < k0 + kn:
                    return t[:, k - k0, :]
            raise AssertionError(k)

        # N * w1t halves for the fp32r diagonal-centering correction term
        nw1t = consts.tile([CH, 2, C], F16, name="nw1t", tag="nw1t")
        nc.vector.tensor_scalar(
            nw1t[:, :, :], wsb[:, 0:4:2, :].bitcast(F32), float(N_), None,
            op0=mybir.AluOpType.mult,
        )

        # stash: rotating [128, cb, C+2] tiles; ones-columns written once.
        stash = [
            small.tile([128, cb, C + 2], F16, name=f"xts{b}", tag=f"xts{b}")
            for b in range(stash_bufs)
        ]
        for b in range(stash_bufs):
            nc.vector.memset(stash[b][:, :, C:C + 2], 1.0)

        # ---- Phase A: G = xf xf^T (+ s columns), symmetric ----
        # Manual PSUM pool lifetimes (pools grab their full complement at
        # the alloc boundary): pg(2)+pt(6) = 8 banks during the Gram; pt
        # releases before pa(5) allocates; pg releases before the algebra.
        g_sb = [small.tile([CH, C + 2], F16, name=f"gsb{h}", tag=f"gsb{h}") for h in range(2)]
        if True:
            pa_att = tc.alloc_tile_pool(name="psum_att", bufs=1, space="PSUM")
            pg = tc.alloc_tile_pool(name="psum_g", bufs=1, space="PSUM")
            g0 = pg.tile([CH, C + 2], F32, name="g0", tag="g0")
            g1 = pg.tile([CH, CH + 2], F32, name="g1", tag="g1")

            def gram_xt(k):
                xts = xt_slice(k)
                nc.tensor.matmul(
                    g0[:], xts[:, 0:CH], xts[:, :],
                    start=(k == 0), stop=False,
                )
                nc.tensor.matmul(
                    g1[:], xts[:, CH:C], xts[:, CH:C + 2],
                    start=(k == 0), stop=False,
                )

            pt = tc.alloc_tile_pool(name="psum_t", bufs=tpsum_bufs, space="PSUM")

            def trans_only(grp):
                """Transpose cb subtiles into PSUM and kick the stash copy."""
                tp = pt.tile([128, cb, C], F32, name="tps", tag="tps")
                for kk in range(cb):
                    ns = grp * cb + kk
                    for h in range(2):
                        nc.tensor.matmul(
                            tp[:, kk, h * CH:(h + 1) * CH],
                            xf_slice(h, ns * 128, 128),
                            ident[:],
                            start=True, stop=True,
                        )
                st = stash[grp % stash_bufs]
                if grp % 2 == 1:
                    nc.scalar.copy(st[:, :, 0:C], tp[:, :, :])
                else:
                    nc.vector.tensor_copy(st[:, :, 0:C], tp[:, :, :])

            def gram_stash(grp):
                st = stash[grp % stash_bufs]
                for kk in range(cb):
                    ns = grp * cb + kk
                    last = ns == PE_SUBS - 1
                    nc.tensor.matmul(
                        g0[:], st[:, kk, 0:CH], st[:, kk, :],
                        start=False, stop=last,
                    )
                    nc.tensor.matmul(
                        g1[:], st[:, kk, CH:C], st[:, kk, CH:C + 2],
                        start=False, stop=last,
                    )

            # software-pipelined fine interleave: per unit emit
            # [xt-half grams, transposes(g), xt-half, stash-grams(g-1)] so
            # the PE never head-blocks on a stash copy (it has ~1.3us of
            # other work queued between a group's copy and its grams).
            xt_done = 0
            pending = None
            for unit, kn in enumerate(XT_DMA_SPLIT):
                ka = kn // 2
                for k in range(xt_done, xt_done + ka):
                    gram_xt(k)
                trans_only(unit)
                for k in range(xt_done + ka, xt_done + kn):
                    gram_xt(k)
                if pending is not None:
                    gram_stash(pending)
                pending = unit
                xt_done += kn
            gram_stash(pending)

            pt.release()
            # att banks live in pa_att (below pg on the pool stack);
            # rank-1 terms fill the PE gap while Vector runs the
            # centering copies.
            att_ps = [pa_att.tile([CH, C], F32, name=f"att{o}", tag=f"att{o}") for o in range(2)]
            for o in range(2):
                osl = slice(o * CH, (o + 1) * CH)
                for h in range(2):
                    nc.tensor.matmul(
                        att_ps[o][:], nw1t[:, h, osl], w2_sb[h],
                        start=(h == 0), stop=False,
                    )

            # centering: cheap s-column copies first (unblock w12s/G10),
            # then the centered copies.  G' = G - N*I.
            nc.vector.tensor_copy(g_sb[0][:, CH:C + 2], g0[:, CH:C + 2])
            nc.vector.tensor_copy(g_sb[1][:, C:C + 2], g1[:, CH:CH + 2])
            nc.vector.scalar_tensor_tensor(
                g_sb[0][:, 0:CH], ident_f[:], -float(N_), g0[:, 0:CH],
                op0=mybir.AluOpType.mult, op1=mybir.AluOpType.add,
            )
            nc.vector.scalar_tensor_tensor(
                g_sb[1][:, CH:C], ident_f[:], -float(N_), g1[:, 0:CH],
                op0=mybir.AluOpType.mult, op1=mybir.AluOpType.add,
            )
            pg.release()
            pa = tc.alloc_tile_pool(name="psum_alg", bufs=1, space="PSUM")

            # ---- C x C algebra (pg released; pa holds w12s/u) ----
            w12s_ps = pa.tile([2, 2 * C], F32, name="w12s", tag="w12s")
            for h in range(2):
                nc.tensor.matmul(
                    w12s_ps[:], g_sb[h][:, C:C + 2], w16[:, 2 * h:2 * h + 2, :],
                    start=(h == 0), stop=(h == 1),
                )
            with tc.tile_pool(name="psum_gt", bufs=1, space="PSUM") as pgt:
                g10 = pgt.tile([128, 128], F32, name="g10", tag="g10")
                nc.tensor.matmul(g10[:], g_sb[0][:, CH:C], ident[:], start=True, stop=True)
                nc.scalar.copy(g_sb[1][:, 0:CH], g10[:])

            w1s_row = small.tile([1, C], F16, name="w1sr", tag="w1sr")
            w2sn_row = small.tile([1, C], F16, name="w2snr", tag="w2snr")
            nc.vector.tensor_copy(w1s_row[:], w12s_ps[0:1, 0:C])
            nc.vector.scalar_tensor_tensor(
                w2sn_row[:], b2_row, float(N),
                w12s_ps[0:1, C:2 * C],
                op0=mybir.AluOpType.mult, op1=mybir.AluOpType.add,
            )

            u_ps = [pa.tile([CH, C], F32, name=f"u{d}", tag=f"u{d}") for d in range(2)]
            for d in range(2):
                for h in range(2):
                    nc.tensor.matmul(
                        u_ps[d][:],
                        g_sb[h][:, d * CH:(d + 1) * CH],
                        w1_sb[h],
                        start=(h == 0), stop=(h == 1),
                    )
            u_sb = [small.tile([CH, C], F16, name=f"usb{d}", tag=f"usb{d}") for d in range(2)]
            for d in range(2):
                nc.vector.tensor_copy(u_sb[d][:], u_ps[d][:])

            # att tails (nw1t terms already accumulated)
            for o in range(2):
                osl = slice(o * CH, (o + 1) * CH)
                for d in range(2):
                    nc.tensor.matmul(
                        att_ps[o][:], u_sb[d][:, osl], w2_sb[d],
                        start=False, stop=False,
                    )
                nc.tensor.matmul(
                    att_ps[o][:], w1s_row[:, osl], b2_row,
                    start=False, stop=False,
                )
                nc.tensor.matmul(
                    att_ps[o][:], b1_row[:, osl], w2sn_row[:],
                    start=False, stop=True,
                )

            # PE keep-warm while softmax(0) runs (reuses the retired w12s
            # bank, WAR-ordered after the two row copies)
            for _ in range(keepwarm):
                nc.tensor.matmul(w12s_ps[:, 0:CH], ident[:, 0:2], ident[:], start=True, stop=True)

            # ---- softmax, folded: A_fin = I + exp(att - max) / rowsum ----
            negmax = [small.tile([CH, 1], F32, name=f"nm{o}", tag=f"nm{o}") for o in range(2)]
            rowsum = [small.tile([CH, 1], F32, name=f"rs{o}", tag=f"rs{o}") for o in range(2)]
            rowinv = [small.tile([CH, 1], F32, name=f"ri{o}", tag=f"ri{o}") for o in range(2)]
            exp_sb = [small.tile([CH, C], F16, name=f"exp{o}", tag=f"exp{o}") for o in range(2)]
            fin_sb = [small.tile([CH, C], F16, name=f"fin{o}", tag=f"fin{o}") for o in range(2)]

            def softmax(o):
                nc.vector.reduce_max(
                    negmax[o][:], att_ps[o][:], axis=mybir.AxisListType.X,
                    negate=True,
                )
                nc.scalar.activation(
                    exp_sb[o][:], att_ps[o][:],
                    mybir.ActivationFunctionType.Exp,
                    bias=negmax[o][:], scale=1.0,
                    accum_out=rowsum[o][:],
                )
                nc.vector.reciprocal(rowinv[o][:], rowsum[o][:])
                nc.vector.scalar_tensor_tensor(
                    fin_sb[o][:], exp_sb[o][:], rowinv[o][:], identI[o][:],
                    op0=mybir.AluOpType.mult, op1=mybir.AluOpType.add,
                )

            softmax(0)
            # a second keep-warm batch covers the softmax(0) latency chain
            for _ in range(keepwarm):
                nc.tensor.matmul(w12s_ps[:, 0:CH], ident[:, 0:2], ident[:], start=True, stop=True)
            softmax(1)
            pa.release()
            pa_att.release()

        # ---- Phase B, pipelined per output half o ----
        # attT(o)[d] = fin_sb[o][:, d-half]^T; y(o) = attT(o)^T @ X.
        assert sum(out_chunks) == N
        ostarts = []
        p_ = 0
        for w_ in out_chunks:
            ostarts.append(p_)
            p_ += w_
        attt_sb = [
            small.tile([CH, 2, CH], F16, name=f"att_sb{o}", tag=f"att_sb{o}")
            for o in range(2)
        ]
        evac_idx = 0
        with tc.tile_pool(name="psum_tr", bufs=2, space="PSUM") as ptr, \
             tc.tile_pool(name="psum_b", bufs=attv_bufs, space="PSUM") as pb, \
             tc.tile_pool(name="outp", bufs=out_bufs) as op:
            # transpose fin_sb halves -> attT with d on partitions (both
            # output halves up front so the o=1 sweep starts seamlessly)
            for o in range(2):
                tpo = ptr.tile([CH, 2, CH], F32, name="tpo", tag="tpo")
                for d in range(2):
                    nc.tensor.matmul(
                        tpo[:, d, :],
                        fin_sb[o][:, d * CH:(d + 1) * CH],
                        ident[:],
                        start=True, stop=True,
                    )
                nc.scalar.copy(attt_sb[o][:, :, :], tpo[:, :, :])
            for o in range(2):
                for j, oc in enumerate(out_chunks):
                    ob = op.tile([CH, 2048], F16, name="ob", tag="ob")
                    for a0 in range(0, oc, 512):
                        aw = min(512, oc - a0)
                        av = pb.tile([CH, 512], F32, name="av", tag="av")
                        for d in range(2):
                            nc.tensor.matmul(
                                av[:, 0:aw],
                                attt_sb[o][:, d, :],
                                xf_slice(d, ostarts[j] + a0, aw),
                                start=(d == 0), stop=(d == 1),
                            )
                        if evac_idx % 2 == 1:
                            nc.scalar.copy(ob[:, a0:a0 + aw], av[:, 0:aw])
                        else:
                            nc.vector.tensor_copy(ob[:, a0:a0 + aw], av[:, 0:aw])
                        evac_idx += 1
                    nc.sync.dma_start(
                        y[:, o, ostarts[j]:ostarts[j] + oc], ob[:, 0:oc]
                    )

    nc.compile()
    return nc


# ---------------------------------------------------------------------------
# Host-side entry point: shard batch over the 8 NeuronCores, run, gather.
# ---------------------------------------------------------------------------

import numpy as np

_NC_CACHE = {}


def _get_nc():
    if "nc" not in _NC_CACHE:
        _NC_CACHE["nc"] = build_nc()
    return _NC_CACHE["nc"]


def make_in_maps(x, w1, b1, w2, b2):
    """Shard + marshal full inputs into per-core input maps (fp16 x)."""
    x = np.asarray(x)
    B, C_, H, W = x.shape
    N = H * W
    xb16 = x.reshape(B, C_, N).astype(np.float16)
    # [B, C, N] -> [B, 128, 2, N]: partition p holds channels p and p+128
    xb = np.ascontiguousarray(xb16.reshape(B, 2, CH, N).transpose(0, 2, 1, 3))
    # host-transposed tail subtiles with pre-baked ones-columns:
    # xt[b, p, k, c] = x[b, c, PE_SUBS*128 + 128k + p]; c in [C, C+2) -> 1
    n0 = PE_SUBS * 128
    tr = xb16[:, :, n0:].reshape(B, C_, XT_CNT, CH).transpose(0, 3, 2, 1)
    xtp = np.ones((B, CH, XT_CNT, C_ + 2), dtype=np.float16)
    xtp[:, :, :, 0:C_] = tr
    xtp = np.ascontiguousarray(xtp)
    w1t = np.asarray(w1, dtype=np.float32).T
    w2t = np.asarray(w2, dtype=np.float32).T
    wp = np.ascontiguousarray(
        np.stack([w1t[0:CH], w2t[0:CH], w1t[CH:C_], w2t[CH:C_]], axis=1)
    )  # [128, 4, C]
    bpk = np.ascontiguousarray(
        np.stack(
            [np.asarray(b1, np.float32), np.asarray(b2, np.float32)], axis=0
        ).reshape(1, 2, C_)
    )
    return [
        {"x": xb[i], "xt": xtp[i], "wp": wp, "bp": bpk}
        for i in range(B)
    ]


def kernel(x, w1, b1, w2, b2):
    """Channel-attention forward for x:(8,256,128,128); returns same shape.

    Data-parallel over the batch: one batch element per NeuronCore.
    """
    from concourse.bass_utils import run_bass_kernel_spmd

    x = np.asarray(x)
    B, C_, H, W = x.shape
    N = H * W
    nc = _get_nc()
    in_maps = make_in_maps(x, w1, b1, w2, b2)
    res = run_bass_kernel_spmd(nc, in_maps, core_ids=list(range(B)))
    out = np.stack(
        [
            res.results[i]["y"].astype(np.float32).transpose(1, 0, 2).reshape(C_, N)
            for i in range(B)
        ],
        axis=0,
    )
    return out.reshape(B, C_, H, W)
